# revision 1
# baseline (speedup 1.0000x reference)
"""Masked 5x5 conv (PixelCNN 'A' mask) on 8 Trainium2 NeuronCores.

Problem (hardcoded): x[4,192,128,128] f32, weight[384,192,5,5] f32,
bias[384] f32, mask[4,1,128,128] i32.
out = where(window_any(mask), conv(x, weight*maskA) + bias, 0).

The 'A' causal mask keeps 12 of 25 taps: rows kh=0,1 fully, row kh=2 only
kw=0,1 -- i.e. every tap reads the current output row or rows above it.

Sharding: core c = (batch b = c//2, row-half = c%2). Each core computes one
batch's 64 output rows for all 384 out channels (3 M=128 chunks).

Per output tile [128 cout, 4 rows x 128 cols = 512] we accumulate 18 K=128
bf16 matmuls into one PSUM bank:
  - 12 taps x channel-chunk ci[0:128]  (from tile xa)
  - 5 tap-PAIRS x ci[128:192]          (from tile xb: lower 64 partitions =
    ci[128:192] data, upper 64 = same data shifted 1 col, so one K=128
    matmul covers two taps that differ by (0,+1))
  - 1 tap-pair (0,4)+(1,4) x ci[128:192] (tile xc: upper shifted one row)
Epilogue: one DVE scalar_tensor_tensor: out = (psum + bias) * valid.
"""

import numpy as np
import ml_dtypes

import concourse.bass as bass
import concourse.tile as tile
from concourse import mybir
from concourse.bass_utils import run_bass_kernel_spmd

B, CIN, COUT, H, W = 4, 192, 384, 128, 128
KH = KW = 5
PAD = 2
NCORES = 8
HHALF = 64          # output rows per core
NROWS = HHALF + 2   # input rows staged per core (2 above)
WP = W + 4          # padded width
FLAT = NROWS * WP   # 66*132 = 8712
RB = 4              # output rows per block
NBLK = HHALF // RB  # 16 blocks
NFREE = RB * W      # 512 = one PSUM bank of fp32

# Active taps of the 'A' mask, (kh, kw)
TAPS = [(0, 0), (0, 1), (0, 2), (0, 3), (0, 4),
        (1, 0), (1, 1), (1, 2), (1, 3), (1, 4),
        (2, 0), (2, 1)]
# ci[128:192] handled as pairs packed into K=128 matmuls.
# slab xb (upper shifted +1 element = +1 col): pairs differing by (0,1)
PAIRS_XB = [((0, 0), (0, 1)), ((0, 2), (0, 3)),
            ((1, 0), (1, 1)), ((1, 2), (1, 3)), ((2, 0), (2, 1))]
# slab xc (upper shifted +132 elements = +1 row): the leftover pair
PAIR_XC = ((0, 4), (1, 4))

BF16 = ml_dtypes.bfloat16


def _build_program():
    """Raw Bass (no Tile): this walrus build rejects instructions carrying
    more than ~1 embedded sync wait, so all synchronization is standalone
    wait_ge instructions with manually-managed semaphores.

    Schedule (per core, ~210us):
      - PE pre-warm: 14 dummy matmuls during the initial DMA wait flip the
        HAM clock gate to 2.4 GHz before the real stream begins.
      - Input DMAs stream in prioritized serialized waves (queues are
        ~45-90 GB/s each, ~358 GB/s aggregate).
      - Phase A runs the 12 xa-slots of tiles 0..7 as soon as the first
        weight/xa chunks land; phase B completes those tiles with the
        xb/xc pair slots once those tensors arrive; then steady state:
        18 K=128 matmuls per [128 cout x 512 spatial] PSUM tile.
      - DVE fuses (psum + bias) * valid into one scalar_tensor_tensor per
        tile, writing a bf16 staging buffer; outputs stream out in 2-tile
        chunks with a tapered, 2-way-split final chunk."""
    nc = bass.Bass()
    bf = mybir.dt.bfloat16
    f32 = mybir.dt.float32

    xa_d = nc.dram_tensor("xa", [128, FLAT], bf, kind="ExternalInput")
    xb_d = nc.dram_tensor("xb", [128, FLAT], bf, kind="ExternalInput")
    xc_d = nc.dram_tensor("xc", [128, FLAT], bf, kind="ExternalInput")
    wt_d = nc.dram_tensor("wt", [128, 18 * COUT], bf, kind="ExternalInput")
    bt_d = nc.dram_tensor("bt", [128, 3], f32, kind="ExternalInput")
    vt_d = nc.dram_tensor("vt", [128, HHALF * W], bf, kind="ExternalInput")
    out_d = nc.dram_tensor("out", [128, 3 * HHALF * W], bf, kind="ExternalOutput")

    NPS = 8           # psum banks in rotation
    PHA = 8           # tiles 0..PHA-1 run split-phase (xa first, xb/xc later)
    XA1 = 38 * WP     # xa chunk 1 covers input rows 0..37 (output blocks 0..7)
    OCH = 2           # out-DMA granularity: blocks per chunk
    NT = 3 * NBLK     # 48 tiles

    from contextlib import ExitStack
    with ExitStack() as ctx:
        xa_t = ctx.enter_context(nc.sbuf_tensor([128, FLAT], bf))
        xb_t = ctx.enter_context(nc.sbuf_tensor([128, FLAT], bf))
        xc_t = ctx.enter_context(nc.sbuf_tensor([128, FLAT], bf))
        wt_t = ctx.enter_context(nc.sbuf_tensor([128, 18 * COUT], bf))
        bt_t = ctx.enter_context(nc.sbuf_tensor([128, 3], f32))
        vt_t = ctx.enter_context(nc.sbuf_tensor([128, HHALF * W], bf))
        st_t = ctx.enter_context(nc.sbuf_tensor([128, 3 * HHALF * W], bf))
        ps_t = ctx.enter_context(nc.psum_tensor([128, NPS * NFREE], f32))
        da0 = ctx.enter_context(nc.semaphore("da0"))
        da1 = ctx.enter_context(nc.semaphore("da1"))
        da2 = ctx.enter_context(nc.semaphore("da2"))
        db1 = ctx.enter_context(nc.semaphore("db1"))
        db2 = ctx.enter_context(nc.semaphore("db2"))
        dc1 = ctx.enter_context(nc.semaphore("dc1"))
        dc2 = ctx.enter_context(nc.semaphore("dc2"))
        dwt1 = ctx.enter_context(nc.semaphore("dwt1"))
        dwt2 = ctx.enter_context(nc.semaphore("dwt2"))
        drest = ctx.enter_context(nc.semaphore("drest"))
        pes = ctx.enter_context(nc.semaphore("pes"))
        dve = ctx.enter_context(nc.semaphore("dve"))
        dout = ctx.enter_context(nc.semaphore("dout"))
        warm = ctx.enter_context(nc.semaphore("warm"))
        block = ctx.enter_context(nc.Block())
        xa_v = xa_t[:].rearrange("p (r c) -> p r c", c=WP)
        xb_v = xb_t[:].rearrange("p (r c) -> p r c", c=WP)
        xc_v = xc_t[:].rearrange("p (r c) -> p r c", c=WP)

        # (global weight-slot index, view, kh, kw)
        slots_a = [(s, xa_v, kh, kw) for s, (kh, kw) in enumerate(TAPS)]
        slots_bc = [(12 + i, xb_v, ta[0], ta[1])
                    for i, (ta, _tb) in enumerate(PAIRS_XB)]
        slots_bc += [(17, xc_v, PAIR_XC[0][0], PAIR_XC[0][1])]

        def emit_mms(tensor, k, sl, start, stop):
            m, blk = divmod(k, NBLK)
            j0 = blk * RB
            ps = ps_t[:, (k % NPS) * NFREE:(k % NPS + 1) * NFREE]
            n = len(sl)
            for i, (s, view, kh, kw) in enumerate(sl):
                mm = nc.tensor.matmul(
                    ps,
                    wt_t[:, s * COUT + m * 128: s * COUT + (m + 1) * 128],
                    view[:, j0 + kh: j0 + kh + RB, kw: kw + W],
                    start=(start and i == 0),
                    stop=(stop and i == n - 1),
                )
                if stop and i == n - 1:
                    mm.then_inc(pes, 1)

        @block.sync
        def _(sync):
            # DMA queues give ~45-90 GB/s per stream and ~358 GB/s aggregate,
            # so stream in prioritized serialized waves, each wave split
            # across a few queues. Wave 1a covers the very first matmuls.
            WT1 = 12 * COUT   # wt cols for the 12 xa slots
            XA0 = 14 * WP     # xa rows 0..13: blocks 0..2
            def split2(dst, src, lo, hi, sem):
                mid = ((lo + hi) // 2 // 4) * 4
                sync.dma_start(dst[:, lo:mid], src[:, lo:mid]).then_inc(sem, 16)
                sync.dma_start(dst[:, mid:hi], src[:, mid:hi]).then_inc(sem, 16)

            # wt1 in three chunks: per-queue BW (~90 GB/s) makes the largest
            # chunk the wave-1a critical path
            W3 = WT1 // 3 // 4 * 4
            sync.dma_start(wt_t[:, 0:W3], wt_d[:, 0:W3]).then_inc(dwt1, 16)
            sync.dma_start(wt_t[:, W3:2 * W3], wt_d[:, W3:2 * W3]).then_inc(dwt1, 16)
            sync.dma_start(wt_t[:, 2 * W3:WT1], wt_d[:, 2 * W3:WT1]).then_inc(dwt1, 16)
            split2(xa_t, xa_d, 0, XA0, da0)
            sync.wait_ge(dwt1, 48)
            sync.wait_ge(da0, 32)
            split2(xa_t, xa_d, XA0, XA1, da1)
            sync.wait_ge(da1, 32)
            # wave 2: phase-B inputs + DVE epilogue inputs; xb first
            split2(xb_t, xb_d, 0, XA1, db1)
            split2(xc_t, xc_d, 0, XA1, dc1)
            sync.dma_start(wt_t[:, WT1:], wt_d[:, WT1:]).then_inc(dwt2, 16)
            sync.dma_start(bt_t[:], bt_d[:]).then_inc(drest, 16)
            split2(vt_t, vt_d, 0, HHALF * W, drest)
            sync.wait_ge(db1, 32)
            sync.wait_ge(dc1, 32)
            # wave 3: steady-state remainders
            split2(xa_t, xa_d, XA1, FLAT, da2)
            split2(xb_t, xb_d, XA1, FLAT, db2)
            split2(xc_t, xc_d, XA1, FLAT, dc2)
            # output chunks of OCH tiles; the last two tiles go out singly
            # (the final DMA is the only one on the critical path, so the
            # smaller and more parallel it is, the shorter the tail)
            nch = NT // OCH
            ninc = 0
            for c in range(nch):
                lo, hi = c * OCH * NFREE, (c + 1) * OCH * NFREE
                if c == nch - 1:
                    # tile 46, then the two halves of the split tile 47
                    sync.wait_ge(dve, NT - 1)
                    mid = lo + NFREE
                    sync.dma_start(out_d[:, lo:mid], st_t[:, lo:mid]).then_inc(dout, 16)
                    mid2 = mid + NFREE // 2
                    sync.wait_ge(dve, NT)
                    sync.dma_start(out_d[:, mid:mid2], st_t[:, mid:mid2]).then_inc(dout, 16)
                    sync.wait_ge(dve, NT + 1)
                    split2(out_d, st_t, mid2, hi, dout)
                    ninc += 4
                else:
                    sync.wait_ge(dve, OCH * (c + 1))
                    sync.dma_start(out_d[:, lo:hi], st_t[:, lo:hi]).then_inc(dout, 16)
                    ninc += 1
            sync.wait_ge(dout, 16 * ninc)

        @block.tensor
        def _(tensor):
            # pre-warm the PE HAM clock gate during the initial DMA wait:
            # ~5us of dummy matmuls (zeros into bank 7, which tile 7
            # later clears with start=True) flips the PE to full clock
            # before the real stream begins. st_t is idle SBUF.
            # 12 dummies x ~426ns cold = ~5us: ends about when wave-1 DMA
            # lands, and >3.4us of PE activity flips the clock to 2.4GHz
            tensor.wait_ge(warm, 1)
            for _ in range(11):
                nc.tensor.matmul(
                    ps_t[:, 7 * NFREE:8 * NFREE],
                    st_t[0:1, 0:128],
                    st_t[0:1, 0:NFREE],
                    start=True,
                    stop=True,
                )
            # phase A: xa-only accumulation for tiles 0..PHA-1, gated on the
            # just-in-time xa row chunks
            tensor.wait_ge(dwt1, 48)
            tensor.wait_ge(da0, 32)
            for k in range(3):
                emit_mms(tensor, k, slots_a, start=True, stop=False)
            tensor.wait_ge(da1, 32)
            for k in range(3, PHA):
                emit_mms(tensor, k, slots_a, start=True, stop=False)
            # phase B: finish tiles 0..PHA-1 with the xb/xc pair slots
            tensor.wait_ge(dwt2, 16)
            tensor.wait_ge(db1, 32)
            tensor.wait_ge(dc1, 32)
            for k in range(PHA):
                emit_mms(tensor, k, slots_bc, start=False, stop=True)
            # steady state
            tensor.wait_ge(da2, 32)
            tensor.wait_ge(db2, 32)
            tensor.wait_ge(dc2, 32)
            # one bank-reuse wait covers 4 tiles: tiles k..k+3 need at most
            # dve >= k+3-(NPS-1) = k-4, and DVE lags PE by well under the
            # 3-tile slack this leaves. Fewer waits = fewer PE queue stalls.
            for k in range(PHA, NT - 1):
                if (k - PHA) % 4 == 0:
                    tensor.wait_ge(dve, min(k + 3, NT - 1) - NPS + 1)
                emit_mms(tensor, k, slots_a, start=True, stop=False)
                emit_mms(tensor, k, slots_bc, start=False, stop=True)
            # final tile split into two 2-row groups (N=256 in half banks):
            # the first half's epilogue+DMA overlaps the second half's
            # matmuls, shortening the kernel tail
            k = NT - 1
            m, blk = divmod(k, NBLK)
            j0 = blk * RB
            for h in range(2):
                # halves in DIFFERENT banks (7, then 6): DVE reads half 1
                # while PE accumulates half 2, and same-bank PE-write +
                # DVE-read is a fatal PSUM collision. Bank 6 (tile 46) is
                # free once dve >= NT-1.
                if h == 1:
                    tensor.wait_ge(dve, NT - 1)
                ps_h = ps_t[:, (7 - h) * NFREE:(7 - h) * NFREE + NFREE // 2]
                for sl, is_last in ((slots_a, False), (slots_bc, True)):
                    n = len(sl)
                    for i, (s, view, kh, kw) in enumerate(sl):
                        mm = nc.tensor.matmul(
                            ps_h,
                            wt_t[:, s * COUT + m * 128: s * COUT + (m + 1) * 128],
                            view[:, j0 + 2 * h + kh: j0 + 2 * h + kh + RB // 2,
                                 kw: kw + W],
                            start=(sl is slots_a and i == 0),
                            stop=(is_last and i == n - 1),
                        )
                        if is_last and i == n - 1:
                            mm.then_inc(pes, 1)

        @block.vector
        def _(vector):
            nc.vector.memset(st_t[0:1, 0:NFREE], 0.0).then_inc(warm, 1)
            vector.wait_ge(drest, 48)  # bias + valid resident (3 chunks)
            for k in range(NT - 1):
                m, blk = divmod(k, NBLK)
                ps = ps_t[:, (k % NPS) * NFREE:(k % NPS + 1) * NFREE]
                vector.wait_ge(pes, k + 1)
                nc.vector.scalar_tensor_tensor(
                    st_t[:, k * NFREE:(k + 1) * NFREE],
                    ps,
                    bt_t[:, m:m + 1],
                    vt_t[:, blk * NFREE:(blk + 1) * NFREE],
                    mybir.AluOpType.add,
                    mybir.AluOpType.mult,
                ).then_inc(dve, 1)
            # final tile: two half-width epilogues matching the split groups
            k = NT - 1
            m, blk = divmod(k, NBLK)
            HF = NFREE // 2
            for h in range(2):
                ps_h = ps_t[:, (7 - h) * NFREE:(7 - h) * NFREE + HF]
                vector.wait_ge(pes, k + 1 + h)
                nc.vector.scalar_tensor_tensor(
                    st_t[:, k * NFREE + h * HF:k * NFREE + (h + 1) * HF],
                    ps_h,
                    bt_t[:, m:m + 1],
                    vt_t[:, blk * NFREE + h * HF:blk * NFREE + (h + 1) * HF],
                    mybir.AluOpType.add,
                    mybir.AluOpType.mult,
                ).then_inc(dve, 1)
    return nc


def _causal_mask():
    m = np.ones((KH, KW), dtype=np.float32)
    m[KH // 2, KW // 2:] = 0.0
    m[KH // 2 + 1:, :] = 0.0
    return m


def _prepare_in_maps(x, weight, bias, mask):
    # window-any of mask -> valid [B, H, W] float32
    ind = (np.asarray(mask)[:, 0] != 0)
    indp = np.zeros((B, H + 2 * PAD, W + 2 * PAD), dtype=bool)
    indp[:, PAD:PAD + H, PAD:PAD + W] = ind
    valid = np.zeros((B, H, W), dtype=bool)
    for dh in range(KH):
        for dw in range(KW):
            valid |= indp[:, dh:dh + H, dw:dw + W]
    valid_f = valid.astype(np.float32)

    w_bf = (np.asarray(weight, dtype=np.float32) * _causal_mask()[None, None]).astype(BF16)

    # 18 weight tiles [K=128, M=384] -> one SBUF image [128, 18, 384]
    wt = np.zeros((18, 128, COUT), dtype=BF16)
    for s, (kh, kw) in enumerate(TAPS):
        wt[s] = w_bf[:, 0:128, kh, kw].T
    for i, (ta, tb) in enumerate(PAIRS_XB):
        wt[12 + i, 0:64] = w_bf[:, 128:192, ta[0], ta[1]].T
        wt[12 + i, 64:128] = w_bf[:, 128:192, tb[0], tb[1]].T
    ta, tb = PAIR_XC
    wt[17, 0:64] = w_bf[:, 128:192, ta[0], ta[1]].T
    wt[17, 64:128] = w_bf[:, 128:192, tb[0], tb[1]].T
    wt_sb = np.ascontiguousarray(wt.transpose(1, 0, 2))

    bias_t = np.ascontiguousarray(
        np.asarray(bias, dtype=np.float32).reshape(3, 128).T)

    x_bf = np.asarray(x, dtype=np.float32).astype(BF16)

    in_maps = []
    for c in range(NCORES):
        b, half = c // 2, c % 2
        r0 = half * HHALF
        xp = np.zeros((CIN, NROWS, WP), dtype=BF16)
        lo = r0 - PAD
        src_lo = max(lo, 0)
        xp[:, src_lo - lo:, PAD:PAD + W] = x_bf[b, :, src_lo:r0 + HHALF, :]
        xf = xp.reshape(CIN, FLAT)
        x2 = xf[128:192]
        sh1 = np.zeros_like(x2)
        sh1[:, :-1] = x2[:, 1:]
        shr = np.zeros_like(x2)
        shr[:, :-WP] = x2[:, WP:]
        vrow = valid_f[b, r0:r0 + HHALF].reshape(1, HHALF * W).astype(BF16)
        vt = np.ascontiguousarray(np.broadcast_to(vrow, (128, HHALF * W)))
        in_maps.append({
            "xa": np.ascontiguousarray(xf[0:128]),
            "xb": np.ascontiguousarray(np.concatenate([x2, sh1], axis=0)),
            "xc": np.ascontiguousarray(np.concatenate([x2, shr], axis=0)),
            "wt": wt_sb.reshape(128, 18 * COUT),
            "bt": bias_t,
            "vt": vt,
        })
    return in_maps


def _assemble(results):
    out_full = np.zeros((B, COUT, H, W), dtype=np.float32)
    for c in range(NCORES):
        b, half = c // 2, c % 2
        o = np.asarray(results[c]["out"]).astype(np.float32)
        o4 = o.reshape(128, 3, HHALF, W).transpose(1, 0, 2, 3).reshape(COUT, HHALF, W)
        out_full[b, :, half * HHALF:(half + 1) * HHALF, :] = o4
    return out_full


def kernel(x, weight, bias, mask, _trace=False):
    in_maps = _prepare_in_maps(x, weight, bias, mask)
    nc = _build_program()
    res = run_bass_kernel_spmd(nc, in_maps, core_ids=list(range(NCORES)),
                               trace=_trace)
    out = _assemble(res.results)
    if _trace:
        return out, res
    return out



# revision 5
# speedup vs baseline: 1.0973x; 1.0973x over previous
"""Masked 5x5 conv (PixelCNN 'A' mask) on 8 Trainium2 NeuronCores.

Problem (hardcoded): x[4,192,128,128] f32, weight[384,192,5,5] f32,
bias[384] f32, mask[4,1,128,128] i32.
out = where(window_any(mask), conv(x, weight*maskA) + bias, 0).

The 'A' causal mask keeps 12 of 25 taps: rows kh=0,1 fully, row kh=2 only
kw=0,1 -- i.e. every tap reads the current output row or rows above it.

Sharding: core c = (batch b = c//2, row-half = c%2). Each core computes one
batch's 64 output rows for all 384 out channels (3 M=128 chunks).

Per output tile [128 cout, 4 rows x 128 cols = 512] we accumulate 16
matmuls into one PSUM bank (contraction 12 taps x 192 cin = 2304):
  - 8 bf16 taps x ci[0:128]            (tile xa)
  - 2 fp8e4 DoubleRow slots x ci[0:128]: taps (0,j)+(1,j) for j=0,1
    packed as K=256 (two k-tiles) via a [p, t:WP, r:WP, c:1] strided AP
    on the fp8 copy of xa -- each runs in the time of ONE bf16 matmul.
  - 5 bf16 tap-PAIRS x ci[128:192]     (tile xb: lower 64 partitions =
    ci[128:192] data, upper 64 = same data shifted 1 col)
  - 1 bf16 tap-pair (0,4)+(1,4) x ci[128:192] (tile xc: upper shifted 1 row)
All weights are pre-scaled x256 on host (exact in bf16; lifts the fp8
weights out of the e4m3 denormal range). PSUM holds 256*conv; the DVE
epilogue is a plain f32->bf16 copy, and the host applies /256 + bias and
the window-any(mask) zeroing in f32 during assembly.
"""

import numpy as np
import ml_dtypes

import bass_rust
import concourse.bass as bass
from concourse import mybir
from concourse.bass_utils import run_bass_kernel_spmd

B, CIN, COUT, H, W = 4, 192, 384, 128, 128
KH = KW = 5
PAD = 2
NCORES = 8
HHALF = 64          # output rows per core
NROWS = HHALF + 2   # input rows staged per core (2 above)
WP = W + 4          # padded width
FLAT = NROWS * WP   # 66*132 = 8712
RB = 4              # output rows per block
NBLK = HHALF // RB  # 16 blocks
NFREE = RB * W      # 512 = one PSUM bank of fp32
WSCALE = 256.0      # weight pre-scale (power of 2; undone on host)

# bf16 xa taps of the 'A' mask, ci[0:128] (slots 0..7)
TAPS_BF = [(0, 2), (0, 3), (0, 4), (1, 2), (1, 3), (1, 4), (2, 0), (2, 1)]
# fp8 DoubleRow slots: vertical tap pairs (0,j)+(1,j), ci[0:128]
DR_COLS = [0, 1]
# ci[128:192] handled as bf16 pairs packed into K=128 matmuls.
PAIRS_XB = [((0, 0), (0, 1)), ((0, 2), (0, 3)),
            ((1, 0), (1, 1)), ((1, 2), (1, 3)), ((2, 0), (2, 1))]
PAIR_XC = ((0, 4), (1, 4))
NSLOT = 14          # bf16 weight slots per m-chunk: 8 xa + 5 xb + 1 xc

BF16 = ml_dtypes.bfloat16
FP8 = ml_dtypes.float8_e4m3
DRM = mybir.MatmulPerfMode.DoubleRow


def _build_program():
    """Raw Bass (no Tile): this walrus build rejects instructions carrying
    more than ~1 embedded sync wait, so all synchronization is standalone
    wait_ge instructions with manually-managed semaphores.

    Schedule (per core, ~172us):
      - PE pre-warm: 8 dummy matmuls (~3.4us) during the initial DMA wait
        flip the HAM clock gate to 2.4 GHz before the real stream begins.
      - Input DMAs stream in prioritized serialized waves (queues are
        ~45-90 GB/s each, ~300+ GB/s aggregate), m-major weight layout so
        the first wave carries only tile-0's weights.
      - Phase A runs the 10 ci[0:128] slots (8 bf16 + 2 fp8 DR) of tiles
        0..7 as row chunks land; phase B completes those tiles with the
        xb/xc pair slots; then steady state: 16 matmuls per tile.
      - DVE drains each PSUM bank with a plain f32->bf16 copy; outputs
        stream out in 2-tile chunks with a split final chunk."""
    nc = bass.Bass()
    bf = mybir.dt.bfloat16
    f8 = mybir.dt.float8e4

    xa_d = nc.dram_tensor("xa", [128, FLAT], bf, kind="ExternalInput")
    x8_d = nc.dram_tensor("x8", [128, FLAT], f8, kind="ExternalInput")
    xb_d = nc.dram_tensor("xb", [128, FLAT], bf, kind="ExternalInput")
    xc_d = nc.dram_tensor("xc", [128, FLAT], bf, kind="ExternalInput")
    wt_d = nc.dram_tensor("wt", [128, 3 * NSLOT * 128], bf, kind="ExternalInput")
    w8_d = nc.dram_tensor("w8", [128, 3 * 2 * 256], f8, kind="ExternalInput")
    out_d = nc.dram_tensor("out", [128, 3 * HHALF * W], bf, kind="ExternalOutput")

    NPS = 8           # psum banks in rotation
    PHA = 8           # tiles 0..PHA-1 run split-phase (phase A, then B)
    R1A, R1B, R1C = 9, 21, 42   # xa/x8 row-wave boundaries
    OCH = 2           # out-DMA granularity: blocks per chunk
    NT = 3 * NBLK     # 48 tiles
    WA = 8 * 128      # cols of the 8 phase-A bf16 slots of one m-chunk

    from contextlib import ExitStack
    with ExitStack() as ctx:
        xa_t = ctx.enter_context(nc.sbuf_tensor([128, FLAT], bf))
        x8_t = ctx.enter_context(nc.sbuf_tensor([128, FLAT], f8))
        xb_t = ctx.enter_context(nc.sbuf_tensor([128, FLAT], bf))
        xc_t = ctx.enter_context(nc.sbuf_tensor([128, FLAT], bf))
        wt_t = ctx.enter_context(nc.sbuf_tensor([128, 3 * NSLOT * 128], bf))
        w8_t = ctx.enter_context(nc.sbuf_tensor([128, 3 * 2 * 256], f8))
        st_t = ctx.enter_context(nc.sbuf_tensor([128, 3 * HHALF * W], bf))
        ps_t = ctx.enter_context(nc.psum_tensor([128, NPS * NFREE], mybir.dt.float32))
        dwa = ctx.enter_context(nc.semaphore("dwa"))
        dxa0 = ctx.enter_context(nc.semaphore("dxa0"))
        dxa1 = ctx.enter_context(nc.semaphore("dxa1"))
        dxa2 = ctx.enter_context(nc.semaphore("dxa2"))
        dxb = ctx.enter_context(nc.semaphore("dxb"))
        dxr = ctx.enter_context(nc.semaphore("dxr"))
        dwtr = ctx.enter_context(nc.semaphore("dwtr"))
        pes = ctx.enter_context(nc.semaphore("pes"))
        dve = ctx.enter_context(nc.semaphore("dve"))
        dout = ctx.enter_context(nc.semaphore("dout"))
        warm = ctx.enter_context(nc.semaphore("warm"))
        block = ctx.enter_context(nc.Block())
        xa_v = xa_t[:].rearrange("p (r c) -> p r c", c=WP)
        xb_v = xb_t[:].rearrange("p (r c) -> p r c", c=WP)
        xc_v = xc_t[:].rearrange("p (r c) -> p r c", c=WP)

        def wt_ap(m, s):
            lo = (m * NSLOT + s) * 128
            return wt_t[:, lo:lo + 128]

        def w8_ap(m, d):
            lo = (m * 2 + d) * 256
            return w8_t[:, lo:lo + 256].rearrange("p (t q) -> p t q", t=2)

        def dr_mv(j0, j, h0=0, rsz=RB):
            # moving AP [p, t(2):WP, r(rsz):WP, c(W):1] at row j0+h0, col j
            return bass_rust.AP(
                x8_t[:].tensor, (j0 + h0) * WP + j,
                bass_rust.VecI64Pair(
                    [[FLAT, 128], [WP, 2], [WP, rsz], [1, W]]))

        def emit_a(tensor, k, start):
            # phase-A slots: 8 bf16 xa taps + 2 fp8 DR pairs
            m, blk = divmod(k, NBLK)
            j0 = blk * RB
            ps = ps_t[:, (k % NPS) * NFREE:(k % NPS + 1) * NFREE]
            for s, (kh, kw) in enumerate(TAPS_BF):
                nc.tensor.matmul(
                    ps, wt_ap(m, s),
                    xa_v[:, j0 + kh: j0 + kh + RB, kw: kw + W],
                    start=(start and s == 0), stop=False)
            for d in DR_COLS:
                nc.tensor.matmul(ps, w8_ap(m, d), dr_mv(j0, d),
                                 start=False, stop=False, perf_mode=DRM)

        def emit_b(tensor, k, stop):
            # phase-B slots: 5 xb pairs + 1 xc pair (ci 128:192)
            m, blk = divmod(k, NBLK)
            j0 = blk * RB
            ps = ps_t[:, (k % NPS) * NFREE:(k % NPS + 1) * NFREE]
            for i, (ta, _tb) in enumerate(PAIRS_XB):
                nc.tensor.matmul(
                    ps, wt_ap(m, 8 + i),
                    xb_v[:, j0 + ta[0]: j0 + ta[0] + RB, ta[1]: ta[1] + W],
                    start=False, stop=False)
            mm = nc.tensor.matmul(
                ps, wt_ap(m, 13),
                xc_v[:, j0 + PAIR_XC[0][0]: j0 + PAIR_XC[0][0] + RB,
                     PAIR_XC[0][1]: PAIR_XC[0][1] + W],
                start=False, stop=stop)
            if stop:
                mm.then_inc(pes, 1)

        @block.sync
        def _(sync):
            def split2(dst, src, lo, hi, sem):
                mid = ((lo + hi) // 2 // 4) * 4
                sync.dma_start(dst[:, lo:mid], src[:, lo:mid]).then_inc(sem, 16)
                sync.dma_start(dst[:, mid:hi], src[:, mid:hi]).then_inc(sem, 16)

            # W1a: tile 0/1 gate -- m0 A-weights, all fp8 weights, rows 0..8
            split2(wt_t, wt_d, 0, WA, dwa)
            split2(w8_t, w8_d, 0, 3 * 2 * 256, dwa)
            split2(xa_t, xa_d, 0, R1A * WP, dxa0)
            sync.dma_start(x8_t[:, 0:R1A * WP], x8_d[:, 0:R1A * WP]).then_inc(dxa0, 16)
            sync.wait_ge(dwa, 64)
            sync.wait_ge(dxa0, 48)
            # W1b1: rows 9..20 (tiles 2..4 phase A)
            split2(xa_t, xa_d, R1A * WP, R1B * WP, dxa1)
            sync.dma_start(x8_t[:, R1A * WP:R1B * WP],
                           x8_d[:, R1A * WP:R1B * WP]).then_inc(dxa1, 16)
            sync.wait_ge(dxa1, 48)
            # W1b2: rows 21..41 (tiles 5..7 phase A)
            split2(xa_t, xa_d, R1B * WP, R1C * WP, dxa2)
            sync.dma_start(x8_t[:, R1B * WP:R1C * WP],
                           x8_d[:, R1B * WP:R1C * WP]).then_inc(dxa2, 16)
            sync.wait_ge(dxa2, 48)
            # W2: phase-B inputs for tiles 0..9: xb/xc rows 0..41 + m0 pair wt
            split2(xb_t, xb_d, 0, R1C * WP, dxb)
            split2(xc_t, xc_d, 0, R1C * WP, dxb)
            sync.dma_start(wt_t[:, WA:NSLOT * 128],
                           wt_d[:, WA:NSLOT * 128]).then_inc(dxb, 16)
            sync.wait_ge(dxb, 80)
            # W3a: remaining x rows 42..65 (steady state from tile 8)
            split2(xa_t, xa_d, R1C * WP, FLAT, dxr)
            sync.dma_start(x8_t[:, R1C * WP:FLAT],
                           x8_d[:, R1C * WP:FLAT]).then_inc(dxr, 16)
            split2(xb_t, xb_d, R1C * WP, FLAT, dxr)
            split2(xc_t, xc_d, R1C * WP, FLAT, dxr)
            sync.wait_ge(dxr, 112)
            # W3b: m1/m2 weights (needed from tile 16)
            split2(wt_t, wt_d, NSLOT * 128, 2 * NSLOT * 128, dwtr)
            split2(wt_t, wt_d, 2 * NSLOT * 128, 3 * NSLOT * 128, dwtr)
            # output chunks of OCH tiles; final chunk tapered + split
            nch = NT // OCH
            ninc = 0
            for c in range(nch):
                lo, hi = c * OCH * NFREE, (c + 1) * OCH * NFREE
                if c == nch - 1:
                    sync.wait_ge(dve, NT - 1)
                    mid = lo + NFREE
                    sync.dma_start(out_d[:, lo:mid], st_t[:, lo:mid]).then_inc(dout, 16)
                    mid2 = mid + NFREE // 2
                    sync.wait_ge(dve, NT)
                    sync.dma_start(out_d[:, mid:mid2], st_t[:, mid:mid2]).then_inc(dout, 16)
                    sync.wait_ge(dve, NT + 1)
                    split2(out_d, st_t, mid2, hi, dout)
                    ninc += 4
                else:
                    sync.wait_ge(dve, OCH * (c + 1))
                    sync.dma_start(out_d[:, lo:hi], st_t[:, lo:hi]).then_inc(dout, 16)
                    ninc += 1
            sync.wait_ge(dout, 16 * ninc)

        @block.tensor
        def _(tensor):
            # pre-warm the PE HAM clock gate during the initial DMA wait:
            # 8 dummy matmuls x ~426ns cold = ~3.4us of PE activity flips
            # the clock to 2.4GHz about when wave-1a lands. st_t is idle.
            tensor.wait_ge(warm, 1)
            for _ in range(8):
                nc.tensor.matmul(
                    ps_t[:, 7 * NFREE:8 * NFREE],
                    st_t[0:1, 0:128],
                    st_t[0:1, 0:NFREE],
                    start=True,
                    stop=True,
                )
            # phase A: ci[0:128] slots for tiles 0..PHA-1, gated on the
            # just-in-time xa/x8 row chunks
            tensor.wait_ge(dwa, 64)
            tensor.wait_ge(dxa0, 48)
            for k in range(2):
                emit_a(tensor, k, start=True)
            tensor.wait_ge(dxa1, 48)
            for k in range(2, 5):
                emit_a(tensor, k, start=True)
            tensor.wait_ge(dxa2, 48)
            for k in range(5, PHA):
                emit_a(tensor, k, start=True)
            # phase B: finish tiles 0..PHA-1 with the xb/xc pair slots
            tensor.wait_ge(dxb, 80)
            for k in range(PHA):
                emit_b(tensor, k, stop=True)
            # steady state (tiles 8,9 still read only wave-2 rows <= 41;
            # the W3a gate is needed first by tile 10, W3b by tile 16)
            for k in range(PHA, NT - 1):
                if k == 10:
                    tensor.wait_ge(dxr, 112)
                if k == 16:
                    tensor.wait_ge(dwtr, 64)
                # one bank-reuse wait covers 4 tiles: tiles k..k+3 need at
                # most dve >= k+3-(NPS-1), and DVE lags PE by well under
                # the 3-tile slack this leaves.
                if (k - PHA) % 4 == 0:
                    tensor.wait_ge(dve, min(k + 3, NT - 1) - NPS + 1)
                emit_a(tensor, k, start=True)
                emit_b(tensor, k, stop=True)
            # final tile split into two 2-row groups (N=256 in half banks):
            # the first half's epilogue+DMA overlaps the second half's
            # matmuls, shortening the kernel tail
            k = NT - 1
            m, blk = divmod(k, NBLK)
            j0 = blk * RB
            for h in range(2):
                # halves in DIFFERENT banks (7, then 6): DVE reads half 1
                # while PE accumulates half 2; bank 6 (tile 46) is free
                # once dve >= NT-1.
                if h == 1:
                    tensor.wait_ge(dve, NT - 1)
                ps_h = ps_t[:, (7 - h) * NFREE:(7 - h) * NFREE + NFREE // 2]
                for s, (kh, kw) in enumerate(TAPS_BF):
                    nc.tensor.matmul(
                        ps_h, wt_ap(m, s),
                        xa_v[:, j0 + 2 * h + kh: j0 + 2 * h + kh + RB // 2,
                             kw: kw + W],
                        start=(s == 0), stop=False)
                for d in DR_COLS:
                    nc.tensor.matmul(ps_h, w8_ap(m, d),
                                     dr_mv(j0, d, h0=2 * h, rsz=RB // 2),
                                     start=False, stop=False, perf_mode=DRM)
                for i, (ta, _tb) in enumerate(PAIRS_XB):
                    nc.tensor.matmul(
                        ps_h, wt_ap(m, 8 + i),
                        xb_v[:, j0 + 2 * h + ta[0]: j0 + 2 * h + ta[0] + RB // 2,
                             ta[1]: ta[1] + W],
                        start=False, stop=False)
                mm = nc.tensor.matmul(
                    ps_h, wt_ap(m, 13),
                    xc_v[:, j0 + 2 * h + PAIR_XC[0][0]:
                         j0 + 2 * h + PAIR_XC[0][0] + RB // 2,
                         PAIR_XC[0][1]: PAIR_XC[0][1] + W],
                    start=False, stop=True)
                mm.then_inc(pes, 1)

        @block.vector
        def _(vector):
            nc.vector.memset(st_t[0:1, 0:NFREE], 0.0).then_inc(warm, 1)
            for k in range(NT - 1):
                ps = ps_t[:, (k % NPS) * NFREE:(k % NPS + 1) * NFREE]
                vector.wait_ge(pes, k + 1)
                nc.vector.tensor_copy(
                    st_t[:, k * NFREE:(k + 1) * NFREE], ps).then_inc(dve, 1)
            # final tile: two half-width copies matching the split groups
            k = NT - 1
            HF = NFREE // 2
            for h in range(2):
                ps_h = ps_t[:, (7 - h) * NFREE:(7 - h) * NFREE + HF]
                vector.wait_ge(pes, k + 1 + h)
                nc.vector.tensor_copy(
                    st_t[:, k * NFREE + h * HF:k * NFREE + (h + 1) * HF],
                    ps_h).then_inc(dve, 1)
    return nc


def _causal_mask():
    m = np.ones((KH, KW), dtype=np.float32)
    m[KH // 2, KW // 2:] = 0.0
    m[KH // 2 + 1:, :] = 0.0
    return m


def _prepare_in_maps(x, weight, bias, mask):
    # window-any of mask -> valid [B, H, W]
    ind = (np.asarray(mask)[:, 0] != 0)
    indp = np.zeros((B, H + 2 * PAD, W + 2 * PAD), dtype=bool)
    indp[:, PAD:PAD + H, PAD:PAD + W] = ind
    valid = np.zeros((B, H, W), dtype=bool)
    for dh in range(KH):
        for dw in range(KW):
            valid |= indp[:, dh:dh + H, dw:dw + W]

    w_sc = np.asarray(weight, dtype=np.float32) * _causal_mask()[None, None] * WSCALE
    w_bf = w_sc.astype(BF16)
    w_f8 = w_sc.astype(FP8)

    # bf16 weights: m-major image [128 K, 3 m-chunks x 14 slots x 128 couts]
    wt = np.zeros((3, NSLOT, 128, 128), dtype=BF16)
    for m in range(3):
        cl, ch = m * 128, (m + 1) * 128
        for s, (kh, kw) in enumerate(TAPS_BF):
            wt[m, s] = w_bf[cl:ch, 0:128, kh, kw].T
        for i, (ta, tb) in enumerate(PAIRS_XB):
            wt[m, 8 + i, 0:64] = w_bf[cl:ch, 128:192, ta[0], ta[1]].T
            wt[m, 8 + i, 64:128] = w_bf[cl:ch, 128:192, tb[0], tb[1]].T
        ta, tb = PAIR_XC
        wt[m, 13, 0:64] = w_bf[cl:ch, 128:192, ta[0], ta[1]].T
        wt[m, 13, 64:128] = w_bf[cl:ch, 128:192, tb[0], tb[1]].T
    wt_sb = np.ascontiguousarray(wt.transpose(2, 0, 1, 3)).reshape(128, 3 * NSLOT * 128)

    # fp8 DR weights: [128 K, 3 m x 2 slots x 2 t x 128 couts], t = kh
    w8 = np.zeros((3, 2, 2, 128, 128), dtype=FP8)
    for m in range(3):
        cl, ch = m * 128, (m + 1) * 128
        for d in DR_COLS:
            for t in range(2):
                w8[m, d, t] = w_f8[cl:ch, 0:128, t, d].T
    w8_sb = np.ascontiguousarray(w8.transpose(3, 0, 1, 2, 4)).reshape(128, 3 * 2 * 256)

    x_f32 = np.asarray(x, dtype=np.float32)
    x_bf = x_f32.astype(BF16)
    x_f8 = x_f32.astype(FP8)

    in_maps = []
    for c in range(NCORES):
        b, half = c // 2, c % 2
        r0 = half * HHALF
        lo = r0 - PAD
        src_lo = max(lo, 0)

        xp = np.zeros((128, NROWS, WP), dtype=BF16)
        xp[:, src_lo - lo:, PAD:PAD + W] = x_bf[b, 0:128, src_lo:r0 + HHALF, :]
        xp8 = np.zeros((128, NROWS, WP), dtype=FP8)
        xp8[:, src_lo - lo:, PAD:PAD + W] = x_f8[b, 0:128, src_lo:r0 + HHALF, :]

        x2p = np.zeros((64, NROWS, WP), dtype=BF16)
        x2p[:, src_lo - lo:, PAD:PAD + W] = x_bf[b, 128:192, src_lo:r0 + HHALF, :]
        x2 = x2p.reshape(64, FLAT)
        sh1 = np.zeros_like(x2)
        sh1[:, :-1] = x2[:, 1:]
        shr = np.zeros_like(x2)
        shr[:, :-WP] = x2[:, WP:]
        in_maps.append({
            "xa": np.ascontiguousarray(xp.reshape(128, FLAT)),
            "x8": np.ascontiguousarray(xp8.reshape(128, FLAT)),
            "xb": np.ascontiguousarray(np.concatenate([x2, sh1], axis=0)),
            "xc": np.ascontiguousarray(np.concatenate([x2, shr], axis=0)),
            "wt": wt_sb,
            "w8": w8_sb,
        })
    return in_maps, valid


def _assemble(results, valid, bias):
    bias_f = np.asarray(bias, dtype=np.float32)
    out_full = np.empty((B, COUT, H, W), dtype=np.float32)
    inv = np.float32(1.0 / WSCALE)
    for c in range(NCORES):
        b, half = c // 2, c % 2
        o = np.asarray(results[c]["out"]).astype(np.float32)
        o4 = o.reshape(128, 3, HHALF, W).transpose(1, 0, 2, 3).reshape(COUT, HHALF, W)
        o4 = o4 * inv + bias_f[:, None, None]
        v = valid[b, half * HHALF:(half + 1) * HHALF, :]
        out_full[b, :, half * HHALF:(half + 1) * HHALF, :] = np.where(v[None], o4, 0.0)
    return out_full


def kernel(x, weight, bias, mask, _trace=False):
    in_maps, valid = _prepare_in_maps(x, weight, bias, mask)
    nc = _build_program()
    res = run_bass_kernel_spmd(nc, in_maps, core_ids=list(range(NCORES)),
                               trace=_trace)
    out = _assemble(res.results, valid, bias)
    if _trace:
        return out, res
    return out


# revision 9
# speedup vs baseline: 1.1163x; 1.0173x over previous
"""Masked 5x5 conv (PixelCNN 'A' mask) on 8 Trainium2 NeuronCores.

Problem (hardcoded): x[4,192,128,128] f32, weight[384,192,5,5] f32,
bias[384] f32, mask[4,1,128,128] i32.
out = where(window_any(mask), conv(x, weight*maskA) + bias, 0).

The 'A' causal mask keeps 12 of 25 taps: rows kh=0,1 fully, row kh=2 only
kw=0,1 -- i.e. every tap reads the current output row or rows above it.

Sharding: core c = (batch b = c//2, row-half = c%2). Each core computes one
batch's 64 output rows for all 384 out channels (3 M=128 chunks).

Per output tile [128 cout, 4 rows x 128 cols = 512] we accumulate 16
matmuls into one PSUM bank (contraction 12 taps x 192 cin = 2304):
  - 8 bf16 taps x ci[0:128]            (tile xa)
  - 2 fp8e4 DoubleRow slots x ci[0:128]: taps (0,j)+(1,j) for j=0,1
    packed as K=256 (two k-tiles) via a [p, t:WP, r:WP, c:1] strided AP
    on the fp8 copy of xa -- each runs in the time of ONE bf16 matmul.
  - 5 bf16 tap-PAIRS x ci[128:192]     (tile xb: lower 64 partitions =
    ci[128:192] data, upper 64 = same data shifted 1 col)
  - 1 bf16 tap-pair (0,4)+(1,4) x ci[128:192] (tile xc: upper shifted 1 row)
All weights are pre-scaled x256 on host (exact in bf16; lifts the fp8
weights out of the e4m3 denormal range). PSUM holds 256*conv; the DVE
epilogue is a plain f32->bf16 copy, and the host applies /256 + bias and
the window-any(mask) zeroing in f32 during assembly.
"""

import numpy as np
import ml_dtypes

import bass_rust
import concourse.bass as bass
from concourse import mybir
from concourse.bass_utils import run_bass_kernel_spmd

B, CIN, COUT, H, W = 4, 192, 384, 128, 128
KH = KW = 5
PAD = 2
NCORES = 8
HHALF = 64          # output rows per core
NROWS = HHALF + 2   # input rows staged per core (2 above)
WP = W + 4          # padded width
FLAT = NROWS * WP   # 66*132 = 8712
RB = 4              # output rows per block
NBLK = HHALF // RB  # 16 blocks
NFREE = RB * W      # 512 = one PSUM bank of fp32
WSCALE = 256.0      # weight pre-scale (power of 2; undone on host)

# bf16 xa taps of the 'A' mask, ci[0:128] (slots 0..7)
TAPS_BF = [(0, 2), (0, 3), (0, 4), (1, 2), (1, 3), (1, 4), (2, 0), (2, 1)]
# fp8 DoubleRow slots: vertical tap pairs (0,j)+(1,j), ci[0:128]
DR_COLS = [0, 1]
# ci[128:192] handled as bf16 pairs packed into K=128 matmuls.
PAIRS_XB = [((0, 0), (0, 1)), ((0, 2), (0, 3)),
            ((1, 0), (1, 1)), ((1, 2), (1, 3)), ((2, 0), (2, 1))]
PAIR_XC = ((0, 4), (1, 4))
NSLOT = 14          # bf16 weight slots per m-chunk: 8 xa + 5 xb + 1 xc

BF16 = ml_dtypes.bfloat16
FP8 = ml_dtypes.float8_e4m3
DRM = mybir.MatmulPerfMode.DoubleRow


def _build_program():
    """Raw Bass (no Tile): this walrus build rejects instructions carrying
    more than ~1 embedded sync wait, so all synchronization is standalone
    wait_ge instructions with manually-managed semaphores.

    Schedule (per core, ~180us):
      - The framework preamble holds every engine until ~8us; the first
        DMA cannot issue before ~7.5us and each dma_start costs ~0.6us of
        Sync issue time, so wave 1 is kept to 7 streams.
      - PE pre-warm: 11 dummy matmuls (~4.7us at the cold clock) end
        about when wave 1 lands, flipping the HAM clock gate to full
        speed with no idle gap (a >2us PE gap resets the clock).
      - No phases: tile k is a contiguous 16-matmul group gated on
        interleaved row-chunks of all four x slabs, so the PE never
        waits mid-tile and the clock never drops.
      - DVE drains each PSUM bank with a plain f32->bf16 copy; outputs
        stream out in 2-tile chunks with a split final chunk."""
    nc = bass.Bass()
    bf = mybir.dt.bfloat16
    f8 = mybir.dt.float8e4

    xa_d = nc.dram_tensor("xa", [128, FLAT], bf, kind="ExternalInput")
    x8_d = nc.dram_tensor("x8", [128, FLAT], f8, kind="ExternalInput")
    xb_d = nc.dram_tensor("xb", [128, FLAT], bf, kind="ExternalInput")
    xc_d = nc.dram_tensor("xc", [128, FLAT], bf, kind="ExternalInput")
    wt_d = nc.dram_tensor("wt", [128, 3 * NSLOT * 128], bf, kind="ExternalInput")
    w8_d = nc.dram_tensor("w8", [128, 3 * 2 * 256], f8, kind="ExternalInput")
    out_d = nc.dram_tensor("out", [128, 3 * HHALF * W], bf, kind="ExternalOutput")

    NPS = 8           # psum banks in rotation
    OCH = 2           # out-DMA granularity: blocks per chunk
    NT = 3 * NBLK     # 48 tiles
    WTM = NSLOT * 128  # wt cols per m-chunk
    # x row-chunk upper bounds (exclusive); tile k reads x rows <= 4*(k%16)+5,
    # so chunk c is first needed by tile FIRST_TILE[c]
    CH = [6, 14, 26, 38, 50, 62, 66]
    FIRST_TILE = [0, 1, 3, 6, 9, 12, 15]

    from contextlib import ExitStack
    with ExitStack() as ctx:
        xa_t = ctx.enter_context(nc.sbuf_tensor([128, FLAT], bf))
        x8_t = ctx.enter_context(nc.sbuf_tensor([128, FLAT], f8))
        xb_t = ctx.enter_context(nc.sbuf_tensor([128, FLAT], bf))
        xc_t = ctx.enter_context(nc.sbuf_tensor([128, FLAT], bf))
        wt_t = ctx.enter_context(nc.sbuf_tensor([128, 3 * NSLOT * 128], bf))
        w8_t = ctx.enter_context(nc.sbuf_tensor([128, 3 * 2 * 256], f8))
        st_t = ctx.enter_context(nc.sbuf_tensor([128, 3 * HHALF * W], bf))
        ps_t = ctx.enter_context(nc.psum_tensor([128, NPS * NFREE], mybir.dt.float32))
        dwt = ctx.enter_context(nc.semaphore("dwt"))
        dx = ctx.enter_context(nc.semaphore("dx"))
        dwtr = ctx.enter_context(nc.semaphore("dwtr"))
        pes = ctx.enter_context(nc.semaphore("pes"))
        dve = ctx.enter_context(nc.semaphore("dve"))
        dout = ctx.enter_context(nc.semaphore("dout"))
        warm = ctx.enter_context(nc.semaphore("warm"))
        block = ctx.enter_context(nc.Block())
        xa_v = xa_t[:].rearrange("p (r c) -> p r c", c=WP)
        xb_v = xb_t[:].rearrange("p (r c) -> p r c", c=WP)
        xc_v = xc_t[:].rearrange("p (r c) -> p r c", c=WP)

        def wt_ap(m, s):
            lo = (m * NSLOT + s) * 128
            return wt_t[:, lo:lo + 128]

        def w8_ap(m, d):
            lo = (m * 2 + d) * 256
            return w8_t[:, lo:lo + 256].rearrange("p (t q) -> p t q", t=2)

        def dr_mv(j0, j, h0=0, rsz=RB):
            # moving AP [p, t(2):WP, r(rsz):WP, c(W):1] at row j0+h0, col j
            return bass_rust.AP(
                x8_t[:].tensor, (j0 + h0) * WP + j,
                bass_rust.VecI64Pair(
                    [[FLAT, 128], [WP, 2], [WP, rsz], [1, W]]))

        def emit_tile(k, half=None):
            # 16 matmuls; half=0/1 emits a 2-row half into a half bank
            m, blk = divmod(k, NBLK)
            j0 = blk * RB
            if half is None:
                h0, rsz = 0, RB
                ps = ps_t[:, (k % NPS) * NFREE:(k % NPS + 1) * NFREE]
            else:
                h0, rsz = 2 * half, RB // 2
                bank = 7 - half
                ps = ps_t[:, bank * NFREE:bank * NFREE + NFREE // 2]
            for s, (kh, kw) in enumerate(TAPS_BF):
                nc.tensor.matmul(
                    ps, wt_ap(m, s),
                    xa_v[:, j0 + h0 + kh: j0 + h0 + kh + rsz, kw: kw + W],
                    start=(s == 0), stop=False)
            for d in DR_COLS:
                nc.tensor.matmul(ps, w8_ap(m, d), dr_mv(j0, d, h0=h0, rsz=rsz),
                                 start=False, stop=False, perf_mode=DRM)
            for i, (ta, _tb) in enumerate(PAIRS_XB):
                nc.tensor.matmul(
                    ps, wt_ap(m, 8 + i),
                    xb_v[:, j0 + h0 + ta[0]: j0 + h0 + ta[0] + rsz,
                         ta[1]: ta[1] + W],
                    start=False, stop=False)
            mm = nc.tensor.matmul(
                ps, wt_ap(m, 13),
                xc_v[:, j0 + h0 + PAIR_XC[0][0]: j0 + h0 + PAIR_XC[0][0] + rsz,
                     PAIR_XC[0][1]: PAIR_XC[0][1] + W],
                start=False, stop=True)
            mm.then_inc(pes, 1)

        @block.sync
        def _(sync):
            def split2(dst, src, lo, hi, sem):
                mid = ((lo + hi) // 2 // 4) * 4
                sync.dma_start(dst[:, lo:mid], src[:, lo:mid]).then_inc(sem, 16)
                sync.dma_start(dst[:, mid:hi], src[:, mid:hi]).then_inc(sem, 16)

            def xchunk(c):
                lo = 0 if c == 0 else CH[c - 1] * WP
                hi = CH[c] * WP
                for dst, src in ((xa_t, xa_d), (xb_t, xb_d), (xc_t, xc_d),
                                 (x8_t, x8_d)):
                    sync.dma_start(dst[:, lo:hi], src[:, lo:hi]).then_inc(dx, 16)

            # wave 1: m0 weights + x rows [0,6) -- the tile-0 gate
            split2(wt_t, wt_d, 0, WTM, dwt)
            sync.dma_start(w8_t[:, 0:512], w8_d[:, 0:512]).then_inc(dwt, 16)
            xchunk(0)
            xchunk(1)
            # keep ~2 chunks in flight: issue chunk c+2 once chunk c landed
            for c in range(2, len(CH)):
                sync.wait_ge(dx, 64 * (c - 1))
                xchunk(c)
            # m1/m2 weights (needed from tile 16)
            split2(wt_t, wt_d, WTM, 2 * WTM, dwtr)
            split2(wt_t, wt_d, 2 * WTM, 3 * WTM, dwtr)
            sync.dma_start(w8_t[:, 512:1536], w8_d[:, 512:1536]).then_inc(dwtr, 16)
            # output chunks of OCH tiles; final chunk tapered + split
            nch = NT // OCH
            ninc = 0
            for c in range(nch):
                lo, hi = c * OCH * NFREE, (c + 1) * OCH * NFREE
                if c == nch - 1:
                    sync.wait_ge(dve, NT - 1)
                    mid = lo + NFREE
                    sync.dma_start(out_d[:, lo:mid], st_t[:, lo:mid]).then_inc(dout, 16)
                    mid2 = mid + NFREE // 2
                    sync.wait_ge(dve, NT)
                    sync.dma_start(out_d[:, mid:mid2], st_t[:, mid:mid2]).then_inc(dout, 16)
                    sync.wait_ge(dve, NT + 1)
                    split2(out_d, st_t, mid2, hi, dout)
                    ninc += 4
                else:
                    sync.wait_ge(dve, OCH * (c + 1))
                    sync.dma_start(out_d[:, lo:hi], st_t[:, lo:hi]).then_inc(dout, 16)
                    ninc += 1
            sync.wait_ge(dout, 16 * ninc)

        @block.tensor
        def _(tensor):
            # pre-warm the PE HAM clock gate during the initial DMA wait:
            # ~11 dummy matmuls x ~427ns cold end about when wave 1 lands;
            # >3.4us of PE activity flips the clock to 2.4GHz. st_t is idle.
            tensor.wait_ge(warm, 1)
            for _ in range(11):
                nc.tensor.matmul(
                    ps_t[:, 7 * NFREE:8 * NFREE],
                    st_t[0:1, 0:128],
                    st_t[0:1, 0:NFREE],
                    start=True,
                    stop=True,
                )
            tensor.wait_ge(dwt, 48)
            gates = {t: 64 * (c + 1) for c, t in enumerate(FIRST_TILE)}
            for k in range(NT - 1):
                if k in gates:
                    tensor.wait_ge(dx, gates[k])
                if k == 16:
                    tensor.wait_ge(dwtr, 80)
                # one bank-reuse wait covers 4 tiles: tiles k..k+3 need at
                # most dve >= k+3-(NPS-1), and DVE lags PE by well under
                # the 3-tile slack this leaves.
                if k >= NPS and (k - NPS) % 4 == 0:
                    tensor.wait_ge(dve, min(k + 3, NT - 1) - NPS + 1)
                emit_tile(k)
            # final tile split into two 2-row groups (N=256 in half banks):
            # the first half's epilogue+DMA overlaps the second half's
            # matmuls, shortening the kernel tail
            for h in range(2):
                # halves in DIFFERENT banks (7, then 6): DVE reads half 1
                # while PE accumulates half 2; bank 6 (tile 46) is free
                # once dve >= NT-1.
                if h == 1:
                    tensor.wait_ge(dve, NT - 1)
                emit_tile(NT - 1, half=h)

        @block.vector
        def _(vector):
            nc.vector.memset(st_t[0:1, 0:NFREE], 0.0).then_inc(warm, 1)
            for k in range(NT - 1):
                ps = ps_t[:, (k % NPS) * NFREE:(k % NPS + 1) * NFREE]
                vector.wait_ge(pes, k + 1)
                nc.vector.tensor_copy(
                    st_t[:, k * NFREE:(k + 1) * NFREE], ps).then_inc(dve, 1)
            # final tile: two half-width copies matching the split groups
            k = NT - 1
            HF = NFREE // 2
            for h in range(2):
                ps_h = ps_t[:, (7 - h) * NFREE:(7 - h) * NFREE + HF]
                vector.wait_ge(pes, k + 1 + h)
                nc.vector.tensor_copy(
                    st_t[:, k * NFREE + h * HF:k * NFREE + (h + 1) * HF],
                    ps_h).then_inc(dve, 1)
    return nc


def _causal_mask():
    m = np.ones((KH, KW), dtype=np.float32)
    m[KH // 2, KW // 2:] = 0.0
    m[KH // 2 + 1:, :] = 0.0
    return m


def _prepare_in_maps(x, weight, bias, mask):
    # window-any of mask -> valid [B, H, W]
    ind = (np.asarray(mask)[:, 0] != 0)
    indp = np.zeros((B, H + 2 * PAD, W + 2 * PAD), dtype=bool)
    indp[:, PAD:PAD + H, PAD:PAD + W] = ind
    valid = np.zeros((B, H, W), dtype=bool)
    for dh in range(KH):
        for dw in range(KW):
            valid |= indp[:, dh:dh + H, dw:dw + W]

    w_sc = np.asarray(weight, dtype=np.float32) * _causal_mask()[None, None] * WSCALE
    w_bf = w_sc.astype(BF16)
    w_f8 = w_sc.astype(FP8)

    # bf16 weights: m-major image [128 K, 3 m-chunks x 14 slots x 128 couts]
    wt = np.zeros((3, NSLOT, 128, 128), dtype=BF16)
    for m in range(3):
        cl, ch = m * 128, (m + 1) * 128
        for s, (kh, kw) in enumerate(TAPS_BF):
            wt[m, s] = w_bf[cl:ch, 0:128, kh, kw].T
        for i, (ta, tb) in enumerate(PAIRS_XB):
            wt[m, 8 + i, 0:64] = w_bf[cl:ch, 128:192, ta[0], ta[1]].T
            wt[m, 8 + i, 64:128] = w_bf[cl:ch, 128:192, tb[0], tb[1]].T
        ta, tb = PAIR_XC
        wt[m, 13, 0:64] = w_bf[cl:ch, 128:192, ta[0], ta[1]].T
        wt[m, 13, 64:128] = w_bf[cl:ch, 128:192, tb[0], tb[1]].T
    wt_sb = np.ascontiguousarray(wt.transpose(2, 0, 1, 3)).reshape(128, 3 * NSLOT * 128)

    # fp8 DR weights: [128 K, 3 m x 2 slots x 2 t x 128 couts], t = kh
    w8 = np.zeros((3, 2, 2, 128, 128), dtype=FP8)
    for m in range(3):
        cl, ch = m * 128, (m + 1) * 128
        for d in DR_COLS:
            for t in range(2):
                w8[m, d, t] = w_f8[cl:ch, 0:128, t, d].T
    w8_sb = np.ascontiguousarray(w8.transpose(3, 0, 1, 2, 4)).reshape(128, 3 * 2 * 256)

    x_f32 = np.asarray(x, dtype=np.float32)
    x_bf = x_f32.astype(BF16)
    x_f8 = x_f32.astype(FP8)

    in_maps = []
    for c in range(NCORES):
        b, half = c // 2, c % 2
        r0 = half * HHALF
        lo = r0 - PAD
        src_lo = max(lo, 0)

        xp = np.zeros((128, NROWS, WP), dtype=BF16)
        xp[:, src_lo - lo:, PAD:PAD + W] = x_bf[b, 0:128, src_lo:r0 + HHALF, :]
        xp8 = np.zeros((128, NROWS, WP), dtype=FP8)
        xp8[:, src_lo - lo:, PAD:PAD + W] = x_f8[b, 0:128, src_lo:r0 + HHALF, :]

        x2p = np.zeros((64, NROWS, WP), dtype=BF16)
        x2p[:, src_lo - lo:, PAD:PAD + W] = x_bf[b, 128:192, src_lo:r0 + HHALF, :]
        x2 = x2p.reshape(64, FLAT)
        sh1 = np.zeros_like(x2)
        sh1[:, :-1] = x2[:, 1:]
        shr = np.zeros_like(x2)
        shr[:, :-WP] = x2[:, WP:]
        in_maps.append({
            "xa": np.ascontiguousarray(xp.reshape(128, FLAT)),
            "x8": np.ascontiguousarray(xp8.reshape(128, FLAT)),
            "xb": np.ascontiguousarray(np.concatenate([x2, sh1], axis=0)),
            "xc": np.ascontiguousarray(np.concatenate([x2, shr], axis=0)),
            "wt": wt_sb,
            "w8": w8_sb,
        })
    return in_maps, valid


def _assemble(results, valid, bias):
    bias_f = np.asarray(bias, dtype=np.float32)
    out_full = np.empty((B, COUT, H, W), dtype=np.float32)
    inv = np.float32(1.0 / WSCALE)
    for c in range(NCORES):
        b, half = c // 2, c % 2
        o = np.asarray(results[c]["out"]).astype(np.float32)
        o4 = o.reshape(128, 3, HHALF, W).transpose(1, 0, 2, 3).reshape(COUT, HHALF, W)
        o4 = o4 * inv + bias_f[:, None, None]
        v = valid[b, half * HHALF:(half + 1) * HHALF, :]
        out_full[b, :, half * HHALF:(half + 1) * HHALF, :] = np.where(v[None], o4, 0.0)
    return out_full


def kernel(x, weight, bias, mask, _trace=False):
    in_maps, valid = _prepare_in_maps(x, weight, bias, mask)
    nc = _build_program()
    res = run_bass_kernel_spmd(nc, in_maps, core_ids=list(range(NCORES)),
                               trace=_trace)
    out = _assemble(res.results, valid, bias)
    if _trace:
        return out, res
    return out


# revision 16
# speedup vs baseline: 1.1258x; 1.0085x over previous
"""Masked 5x5 conv (PixelCNN 'A' mask) on 8 Trainium2 NeuronCores.

Problem (hardcoded): x[4,192,128,128] f32, weight[384,192,5,5] f32,
bias[384] f32, mask[4,1,128,128] i32.
out = where(window_any(mask), conv(x, weight*maskA) + bias, 0).

The 'A' causal mask keeps 12 of 25 taps: rows kh=0,1 fully, row kh=2 only
kw=0,1 -- i.e. every tap reads the current output row or rows above it.

Sharding: core c = (batch b = c//2, row-half = c%2). Each core computes one
batch's 64 output rows for all 384 out channels (3 M=128 chunks).

Per output tile [128 cout, 4 rows x 128 cols = 512] we accumulate 16
matmuls into one PSUM bank (contraction 12 taps x 192 cin = 2304):
  - 8 bf16 taps x ci[0:128]            (tile xa)
  - 2 fp8e4 DoubleRow slots x ci[0:128]: taps (0,j)+(1,j) for j=0,1
    packed as K=256 (two k-tiles) via a [p, t:WP, r:WP, c:1] strided AP
    on the fp8 copy of xa -- each runs in the time of ONE bf16 matmul.
  - 5 bf16 tap-PAIRS x ci[128:192]     (tile xb: lower 64 partitions =
    ci[128:192] data, upper 64 = same data shifted 1 col)
  - 1 bf16 tap-pair (0,4)+(1,4) x ci[128:192] (tile xc: upper shifted 1 row)
All weights are pre-scaled x256 on host (exact in bf16; lifts the fp8
weights out of the e4m3 denormal range). PSUM holds 256*conv; the DVE
epilogue is a plain f32->bf16 copy, and the host applies /256 + bias and
the window-any(mask) zeroing in f32 during assembly.
"""

import numpy as np
import ml_dtypes

import bass_rust
import concourse.bass as bass
from concourse import mybir
from concourse.bass_utils import run_bass_kernel_spmd

B, CIN, COUT, H, W = 4, 192, 384, 128, 128
KH = KW = 5
PAD = 2
NCORES = 8
HHALF = 64          # output rows per core
NROWS = HHALF + 2   # input rows staged per core (2 above)
WP = W + 4          # padded width
FLAT = NROWS * WP   # 66*132 = 8712
RB = 4              # output rows per block
NBLK = HHALF // RB  # 16 blocks
NFREE = RB * W      # 512 = one PSUM bank of fp32
WSCALE = 256.0      # weight pre-scale (power of 2; undone on host)

# bf16 xa taps of the 'A' mask, ci[0:128] (slots 0..7)
TAPS_BF = [(0, 2), (0, 3), (0, 4), (1, 2), (1, 3), (1, 4), (2, 0), (2, 1)]
# fp8 DoubleRow slots: vertical tap pairs (0,j)+(1,j), ci[0:128]
DR_COLS = [0, 1]
# ci[128:192] handled as bf16 pairs packed into K=128 matmuls.
PAIRS_XB = [((0, 0), (0, 1)), ((0, 2), (0, 3)),
            ((1, 0), (1, 1)), ((1, 2), (1, 3)), ((2, 0), (2, 1))]
PAIR_XC = ((0, 4), (1, 4))
NSLOT = 14          # bf16 weight slots per m-chunk: 8 xa + 5 xb + 1 xc

BF16 = ml_dtypes.bfloat16
FP8 = ml_dtypes.float8_e4m3
DRM = mybir.MatmulPerfMode.DoubleRow


def _build_program():
    """Raw Bass (no Tile): this walrus build rejects instructions carrying
    more than ~1 embedded sync wait, so all synchronization is standalone
    wait_ge instructions with manually-managed semaphores.

    Schedule (per core, ~180us):
      - The framework preamble holds every engine until ~8us; the first
        DMA cannot issue before ~7.5us and each dma_start costs ~0.6us of
        Sync issue time, so wave 1 is kept to 7 streams.
      - PE pre-warm: 11 dummy matmuls (~4.7us at the cold clock) end
        about when wave 1 lands, flipping the HAM clock gate to full
        speed with no idle gap (a >2us PE gap resets the clock).
      - No phases: tile k is a contiguous 16-matmul group gated on
        interleaved row-chunks of all four x slabs, so the PE never
        waits mid-tile and the clock never drops.
      - DVE drains each PSUM bank with a plain f32->bf16 copy; outputs
        stream out in 2-tile chunks with a split final chunk."""
    nc = bass.Bass()
    bf = mybir.dt.bfloat16
    f8 = mybir.dt.float8e4

    xa_d = nc.dram_tensor("xa", [128, FLAT], bf, kind="ExternalInput")
    x8_d = nc.dram_tensor("x8", [128, FLAT], f8, kind="ExternalInput")
    xb_d = nc.dram_tensor("xb", [128, FLAT], bf, kind="ExternalInput")
    xc_d = nc.dram_tensor("xc", [128, FLAT], bf, kind="ExternalInput")
    wt_d = nc.dram_tensor("wt", [128, 3 * NSLOT * 128], bf, kind="ExternalInput")
    w8_d = nc.dram_tensor("w8", [128, 3 * 2 * 256], f8, kind="ExternalInput")
    out_d = nc.dram_tensor("out", [128, 3 * HHALF * W], bf, kind="ExternalOutput")

    NPS = 8           # psum banks in rotation
    OCH = 2           # out-DMA granularity: blocks per chunk
    NT = 3 * NBLK     # 48 tiles
    WTM = NSLOT * 128  # wt cols per m-chunk
    # x row-chunk upper bounds (exclusive); tile k reads x rows <= 4*(k%16)+5,
    # so chunk c is first needed by tile FIRST_TILE[c]
    CH = [6, 14, 26, 38, 50, 62, 66]
    FIRST_TILE = [0, 1, 3, 6, 9, 12, 15]

    from contextlib import ExitStack
    with ExitStack() as ctx:
        xa_t = ctx.enter_context(nc.sbuf_tensor([128, FLAT], bf))
        x8_t = ctx.enter_context(nc.sbuf_tensor([128, FLAT], f8))
        xb_t = ctx.enter_context(nc.sbuf_tensor([128, FLAT], bf))
        xc_t = ctx.enter_context(nc.sbuf_tensor([128, FLAT], bf))
        wt_t = ctx.enter_context(nc.sbuf_tensor([128, 3 * NSLOT * 128], bf))
        w8_t = ctx.enter_context(nc.sbuf_tensor([128, 3 * 2 * 256], f8))
        st_t = ctx.enter_context(nc.sbuf_tensor([128, 3 * HHALF * W], bf))
        ps_t = ctx.enter_context(nc.psum_tensor([128, NPS * NFREE], mybir.dt.float32))
        dwt = ctx.enter_context(nc.semaphore("dwt"))
        dxa0 = ctx.enter_context(nc.semaphore("dxa0"))
        dxb0 = ctx.enter_context(nc.semaphore("dxb0"))
        dx = ctx.enter_context(nc.semaphore("dx"))
        dwtr = ctx.enter_context(nc.semaphore("dwtr"))
        pes = ctx.enter_context(nc.semaphore("pes"))
        dve = ctx.enter_context(nc.semaphore("dve"))
        dout = ctx.enter_context(nc.semaphore("dout"))
        warm = ctx.enter_context(nc.semaphore("warm"))
        block = ctx.enter_context(nc.Block())
        xa_v = xa_t[:].rearrange("p (r c) -> p r c", c=WP)
        xb_v = xb_t[:].rearrange("p (r c) -> p r c", c=WP)
        xc_v = xc_t[:].rearrange("p (r c) -> p r c", c=WP)

        def wt_ap(m, s):
            lo = (m * NSLOT + s) * 128
            return wt_t[:, lo:lo + 128]

        def w8_ap(m, d):
            lo = (m * 2 + d) * 256
            return w8_t[:, lo:lo + 256].rearrange("p (t q) -> p t q", t=2)

        def dr_mv(j0, j, h0=0, rsz=RB):
            # moving AP [p, t(2):WP, r(rsz):WP, c(W):1] at row j0+h0, col j
            return bass_rust.AP(
                x8_t[:].tensor, (j0 + h0) * WP + j,
                bass_rust.VecI64Pair(
                    [[FLAT, 128], [WP, 2], [WP, rsz], [1, W]]))

        QBANK = [7, 4, 5, 6]   # final-tile quarter -> psum bank

        def emit_tile_a(k, h0=0, rsz=RB, qb=None):
            # part 1: the 10 ci[0:128] slots (8 bf16 xa + 2 fp8 DR)
            m, blk = divmod(k, NBLK)
            j0 = blk * RB
            if qb is None:
                ps = ps_t[:, (k % NPS) * NFREE:(k % NPS + 1) * NFREE]
            else:
                ps = ps_t[:, qb * NFREE:qb * NFREE + rsz * W]
            for s, (kh, kw) in enumerate(TAPS_BF):
                nc.tensor.matmul(
                    ps, wt_ap(m, s),
                    xa_v[:, j0 + h0 + kh: j0 + h0 + kh + rsz, kw: kw + W],
                    start=(s == 0), stop=False)
            for d in DR_COLS:
                nc.tensor.matmul(ps, w8_ap(m, d), dr_mv(j0, d, h0=h0, rsz=rsz),
                                 start=False, stop=False, perf_mode=DRM)

        def emit_tile_b(k, h0=0, rsz=RB, qb=None):
            # part 2: the 6 ci[128:192] pair slots
            m, blk = divmod(k, NBLK)
            j0 = blk * RB
            if qb is None:
                ps = ps_t[:, (k % NPS) * NFREE:(k % NPS + 1) * NFREE]
            else:
                ps = ps_t[:, qb * NFREE:qb * NFREE + rsz * W]
            for i, (ta, _tb) in enumerate(PAIRS_XB):
                nc.tensor.matmul(
                    ps, wt_ap(m, 8 + i),
                    xb_v[:, j0 + h0 + ta[0]: j0 + h0 + ta[0] + rsz,
                         ta[1]: ta[1] + W],
                    start=False, stop=False)
            mm = nc.tensor.matmul(
                ps, wt_ap(m, 13),
                xc_v[:, j0 + h0 + PAIR_XC[0][0]: j0 + h0 + PAIR_XC[0][0] + rsz,
                     PAIR_XC[0][1]: PAIR_XC[0][1] + W],
                start=False, stop=True)
            mm.then_inc(pes, 1)

        def emit_tile(k):
            emit_tile_a(k)
            emit_tile_b(k)

        @block.sync
        def _(sync):
            def split2(dst, src, lo, hi, sem):
                mid = ((lo + hi) // 2 // 4) * 4
                sync.dma_start(dst[:, lo:mid], src[:, lo:mid]).then_inc(sem, 16)
                sync.dma_start(dst[:, mid:hi], src[:, mid:hi]).then_inc(sem, 16)

            def xchunk(c):
                lo = 0 if c == 0 else CH[c - 1] * WP
                hi = CH[c] * WP
                for dst, src in ((xa_t, xa_d), (xb_t, xb_d), (xc_t, xc_d),
                                 (x8_t, x8_d)):
                    sync.dma_start(dst[:, lo:hi], src[:, lo:hi]).then_inc(dx, 16)

            # wave 1, in issue-priority order (each dma_start costs ~0.6us
            # of Sync time, so order = priority): the m0 weights and the
            # xa/x8 rows [0,6) gate tile 0's first 10 matmuls; the xb/xc
            # rows gate its last 6.
            W3 = WTM // 3 // 4 * 4
            sync.dma_start(wt_t[:, 0:W3], wt_d[:, 0:W3]).then_inc(dwt, 16)
            sync.dma_start(wt_t[:, W3:2 * W3], wt_d[:, W3:2 * W3]).then_inc(dwt, 16)
            sync.dma_start(wt_t[:, 2 * W3:WTM], wt_d[:, 2 * W3:WTM]).then_inc(dwt, 16)
            c0 = CH[0] * WP
            sync.dma_start(xa_t[:, 0:c0], xa_d[:, 0:c0]).then_inc(dxa0, 16)
            sync.dma_start(x8_t[:, 0:c0], x8_d[:, 0:c0]).then_inc(dxa0, 16)
            sync.dma_start(w8_t[:, 0:512], w8_d[:, 0:512]).then_inc(dwt, 16)
            sync.dma_start(xb_t[:, 0:c0], xb_d[:, 0:c0]).then_inc(dxb0, 16)
            sync.dma_start(xc_t[:, 0:c0], xc_d[:, 0:c0]).then_inc(dxb0, 16)
            xchunk(1)
            # keep ~2 chunks in flight: issue chunk c+2 once chunk c landed
            sync.wait_ge(dxb0, 32)
            xchunk(2)
            for c in range(3, len(CH)):
                sync.wait_ge(dx, 64 * (c - 2))
                xchunk(c)
            # m1/m2 weights (needed from tile 16)
            split2(wt_t, wt_d, WTM, 2 * WTM, dwtr)
            split2(wt_t, wt_d, 2 * WTM, 3 * WTM, dwtr)
            sync.dma_start(w8_t[:, 512:1536], w8_d[:, 512:1536]).then_inc(dwtr, 16)
            # output chunks of OCH tiles; final chunk tapered: tile 46
            # whole, then tile 47 in four quarter-DMAs as each quarter's
            # epilogue lands
            nch = NT // OCH
            ninc = 0
            for c in range(nch):
                lo, hi = c * OCH * NFREE, (c + 1) * OCH * NFREE
                if c == nch - 1:
                    sync.wait_ge(dve, NT - 1)
                    mid = lo + NFREE
                    sync.dma_start(out_d[:, lo:mid], st_t[:, lo:mid]).then_inc(dout, 16)
                    QF = NFREE // 4
                    for q in range(4):
                        sync.wait_ge(dve, NT + q)
                        sync.dma_start(out_d[:, mid + q * QF:mid + (q + 1) * QF],
                                       st_t[:, mid + q * QF:mid + (q + 1) * QF]
                                       ).then_inc(dout, 16)
                    ninc += 5
                else:
                    sync.wait_ge(dve, OCH * (c + 1))
                    sync.dma_start(out_d[:, lo:hi], st_t[:, lo:hi]).then_inc(dout, 16)
                    ninc += 1
            sync.wait_ge(dout, 16 * ninc)

        @block.tensor
        def _(tensor):
            # pre-warm the PE HAM clock gate during the initial DMA wait:
            # ~11 dummy matmuls x ~427ns cold end about when wave 1 lands;
            # >3.4us of PE activity flips the clock to 2.4GHz. st_t is idle.
            tensor.wait_ge(warm, 1)
            for _ in range(11):
                nc.tensor.matmul(
                    ps_t[:, 7 * NFREE:8 * NFREE],
                    st_t[0:1, 0:128],
                    st_t[0:1, 0:NFREE],
                    start=True,
                    stop=True,
                )
            # tile 0 split-phase: its first 10 matmuls need only wt/w8 +
            # xa/x8 rows [0,6), which are DMA-issued ahead of xb/xc -- the
            # PE starts ~1.5us earlier and the clock never drops.
            tensor.wait_ge(dwt, 64)
            tensor.wait_ge(dxa0, 32)
            emit_tile_a(0)
            tensor.wait_ge(dxb0, 32)
            emit_tile_b(0)
            gates = {t: 64 * c for c, t in enumerate(FIRST_TILE) if c > 0}
            for k in range(1, NT - 1):
                if k in gates:
                    tensor.wait_ge(dx, gates[k])
                if k == 16:
                    tensor.wait_ge(dwtr, 80)
                # one bank-reuse wait covers 4 tiles: tiles k..k+3 need at
                # most dve >= k+3-(NPS-1), and DVE lags PE by well under
                # the 3-tile slack this leaves.
                if k >= NPS and (k - NPS) % 4 == 0:
                    tensor.wait_ge(dve, min(k + 3, NT - 1) - NPS + 1)
                emit_tile(k)
            # final tile split into four 1-row quarters in four different
            # banks: each quarter's epilogue+out-DMA overlaps the next
            # quarter's matmuls, and the last DMA is only 32KB
            for q in range(4):
                # QBANK[q] is free once its previous tile's copy landed:
                # bank 7 <- tile 39 (dve>=40, covered by the k=44 wait),
                # bank 4/5/6 <- tiles 44/45/46 (dve >= 45/46/47)
                if q > 0:
                    tensor.wait_ge(dve, NT - 4 + q)
                emit_tile_a(NT - 1, h0=q, rsz=1, qb=QBANK[q])
                emit_tile_b(NT - 1, h0=q, rsz=1, qb=QBANK[q])

        @block.vector
        def _(vector):
            nc.vector.memset(st_t[0:1, 0:NFREE], 0.0).then_inc(warm, 1)
            for k in range(NT - 1):
                ps = ps_t[:, (k % NPS) * NFREE:(k % NPS + 1) * NFREE]
                vector.wait_ge(pes, k + 1)
                nc.vector.tensor_copy(
                    st_t[:, k * NFREE:(k + 1) * NFREE], ps).then_inc(dve, 1)
            # final tile: four quarter-width copies matching the split groups
            k = NT - 1
            QF = NFREE // 4
            for q in range(4):
                ps_q = ps_t[:, QBANK[q] * NFREE:QBANK[q] * NFREE + QF]
                vector.wait_ge(pes, k + 1 + q)
                nc.vector.tensor_copy(
                    st_t[:, k * NFREE + q * QF:k * NFREE + (q + 1) * QF],
                    ps_q).then_inc(dve, 1)
    return nc


def _causal_mask():
    m = np.ones((KH, KW), dtype=np.float32)
    m[KH // 2, KW // 2:] = 0.0
    m[KH // 2 + 1:, :] = 0.0
    return m


def _prepare_in_maps(x, weight, bias, mask):
    # window-any of mask -> valid [B, H, W]
    ind = (np.asarray(mask)[:, 0] != 0)
    indp = np.zeros((B, H + 2 * PAD, W + 2 * PAD), dtype=bool)
    indp[:, PAD:PAD + H, PAD:PAD + W] = ind
    valid = np.zeros((B, H, W), dtype=bool)
    for dh in range(KH):
        for dw in range(KW):
            valid |= indp[:, dh:dh + H, dw:dw + W]

    w_sc = np.asarray(weight, dtype=np.float32) * _causal_mask()[None, None] * WSCALE
    w_bf = w_sc.astype(BF16)
    w_f8 = w_sc.astype(FP8)

    # bf16 weights: m-major image [128 K, 3 m-chunks x 14 slots x 128 couts]
    wt = np.zeros((3, NSLOT, 128, 128), dtype=BF16)
    for m in range(3):
        cl, ch = m * 128, (m + 1) * 128
        for s, (kh, kw) in enumerate(TAPS_BF):
            wt[m, s] = w_bf[cl:ch, 0:128, kh, kw].T
        for i, (ta, tb) in enumerate(PAIRS_XB):
            wt[m, 8 + i, 0:64] = w_bf[cl:ch, 128:192, ta[0], ta[1]].T
            wt[m, 8 + i, 64:128] = w_bf[cl:ch, 128:192, tb[0], tb[1]].T
        ta, tb = PAIR_XC
        wt[m, 13, 0:64] = w_bf[cl:ch, 128:192, ta[0], ta[1]].T
        wt[m, 13, 64:128] = w_bf[cl:ch, 128:192, tb[0], tb[1]].T
    wt_sb = np.ascontiguousarray(wt.transpose(2, 0, 1, 3)).reshape(128, 3 * NSLOT * 128)

    # fp8 DR weights: [128 K, 3 m x 2 slots x 2 t x 128 couts], t = kh
    w8 = np.zeros((3, 2, 2, 128, 128), dtype=FP8)
    for m in range(3):
        cl, ch = m * 128, (m + 1) * 128
        for d in DR_COLS:
            for t in range(2):
                w8[m, d, t] = w_f8[cl:ch, 0:128, t, d].T
    w8_sb = np.ascontiguousarray(w8.transpose(3, 0, 1, 2, 4)).reshape(128, 3 * 2 * 256)

    x_f32 = np.asarray(x, dtype=np.float32)
    x_bf = x_f32.astype(BF16)
    x_f8 = x_f32.astype(FP8)

    in_maps = []
    for c in range(NCORES):
        b, half = c // 2, c % 2
        r0 = half * HHALF
        lo = r0 - PAD
        src_lo = max(lo, 0)

        xp = np.zeros((128, NROWS, WP), dtype=BF16)
        xp[:, src_lo - lo:, PAD:PAD + W] = x_bf[b, 0:128, src_lo:r0 + HHALF, :]
        xp8 = np.zeros((128, NROWS, WP), dtype=FP8)
        xp8[:, src_lo - lo:, PAD:PAD + W] = x_f8[b, 0:128, src_lo:r0 + HHALF, :]

        x2p = np.zeros((64, NROWS, WP), dtype=BF16)
        x2p[:, src_lo - lo:, PAD:PAD + W] = x_bf[b, 128:192, src_lo:r0 + HHALF, :]
        x2 = x2p.reshape(64, FLAT)
        sh1 = np.zeros_like(x2)
        sh1[:, :-1] = x2[:, 1:]
        shr = np.zeros_like(x2)
        shr[:, :-WP] = x2[:, WP:]
        in_maps.append({
            "xa": np.ascontiguousarray(xp.reshape(128, FLAT)),
            "x8": np.ascontiguousarray(xp8.reshape(128, FLAT)),
            "xb": np.ascontiguousarray(np.concatenate([x2, sh1], axis=0)),
            "xc": np.ascontiguousarray(np.concatenate([x2, shr], axis=0)),
            "wt": wt_sb,
            "w8": w8_sb,
        })
    return in_maps, valid


def _assemble(results, valid, bias):
    bias_f = np.asarray(bias, dtype=np.float32)
    out_full = np.empty((B, COUT, H, W), dtype=np.float32)
    inv = np.float32(1.0 / WSCALE)
    for c in range(NCORES):
        b, half = c // 2, c % 2
        o = np.asarray(results[c]["out"]).astype(np.float32)
        o4 = o.reshape(128, 3, HHALF, W).transpose(1, 0, 2, 3).reshape(COUT, HHALF, W)
        o4 = o4 * inv + bias_f[:, None, None]
        v = valid[b, half * HHALF:(half + 1) * HHALF, :]
        out_full[b, :, half * HHALF:(half + 1) * HHALF, :] = np.where(v[None], o4, 0.0)
    return out_full


def kernel(x, weight, bias, mask, _trace=False):
    in_maps, valid = _prepare_in_maps(x, weight, bias, mask)
    nc = _build_program()
    res = run_bass_kernel_spmd(nc, in_maps, core_ids=list(range(NCORES)),
                               trace=_trace)
    out = _assemble(res.results, valid, bias)
    if _trace:
        return out, res
    return out


# revision 20
# speedup vs baseline: 1.1333x; 1.0067x over previous
"""Masked 5x5 conv (PixelCNN 'A' mask) on 8 Trainium2 NeuronCores.

Problem (hardcoded): x[4,192,128,128] f32, weight[384,192,5,5] f32,
bias[384] f32, mask[4,1,128,128] i32.
out = where(window_any(mask), conv(x, weight*maskA) + bias, 0).

The 'A' causal mask keeps 12 of 25 taps: rows kh=0,1 fully, row kh=2 only
kw=0,1 -- i.e. every tap reads the current output row or rows above it.

Sharding: core c = (batch b = c//2, row-half = c%2). Each core computes one
batch's 64 output rows for all 384 out channels (3 M=128 chunks).

Per output tile [128 cout, 4 rows x 128 cols = 512] we accumulate 16
matmuls into one PSUM bank (contraction 12 taps x 192 cin = 2304):
  - 8 bf16 taps x ci[0:128]            (tile xa)
  - 2 fp8e4 DoubleRow slots x ci[0:128]: taps (0,j)+(1,j) for j=0,1
    packed as K=256 (two k-tiles) via a [p, t:WP, r:WP, c:1] strided AP
    on the fp8 copy of xa -- each runs in the time of ONE bf16 matmul.
  - 5 bf16 tap-PAIRS x ci[128:192]     (tile xb: lower 64 partitions =
    ci[128:192] data, upper 64 = same data shifted 1 col)
  - 1 bf16 tap-pair (0,4)+(1,4) x ci[128:192] (tile xc: upper shifted 1 row)
All weights are pre-scaled x256 on host (exact in bf16; lifts the fp8
weights out of the e4m3 denormal range). PSUM holds 256*conv; the DVE
epilogue is a plain f32->bf16 copy, and the host applies /256 + bias and
the window-any(mask) zeroing in f32 during assembly.
"""

import numpy as np
import ml_dtypes

import bass_rust
import concourse.bass as bass
from concourse import mybir
from concourse.bass_utils import run_bass_kernel_spmd

B, CIN, COUT, H, W = 4, 192, 384, 128, 128
KH = KW = 5
PAD = 2
NCORES = 8
HHALF = 64          # output rows per core
NROWS = HHALF + 2   # input rows staged per core (2 above)
WP = W + 4          # padded width
FLAT = NROWS * WP   # 66*132 = 8712
RB = 4              # output rows per block
NBLK = HHALF // RB  # 16 blocks
NFREE = RB * W      # 512 = one PSUM bank of fp32
WSCALE = 256.0      # weight pre-scale (power of 2; undone on host)

# bf16 xa taps of the 'A' mask, ci[0:128] (slots 0..7)
TAPS_BF = [(0, 2), (0, 3), (0, 4), (1, 2), (1, 3), (1, 4), (2, 0), (2, 1)]
# fp8 DoubleRow slots: vertical tap pairs (0,j)+(1,j), ci[0:128]
DR_COLS = [0, 1]
# ci[128:192] handled as bf16 pairs packed into K=128 matmuls.
PAIRS_XB = [((0, 0), (0, 1)), ((0, 2), (0, 3)),
            ((1, 0), (1, 1)), ((1, 2), (1, 3)), ((2, 0), (2, 1))]
PAIR_XC = ((0, 4), (1, 4))
NSLOT = 14          # bf16 weight slots per m-chunk: 8 xa + 5 xb + 1 xc

BF16 = ml_dtypes.bfloat16
FP8 = ml_dtypes.float8_e4m3
DRM = mybir.MatmulPerfMode.DoubleRow


def _build_program():
    """Raw Bass (no Tile): this walrus build rejects instructions carrying
    more than ~1 embedded sync wait, so all synchronization is standalone
    wait_ge instructions with manually-managed semaphores.

    Schedule (per core, ~180us):
      - The framework preamble holds every engine until ~8us; the first
        DMA cannot issue before ~7.5us and each dma_start costs ~0.6us of
        Sync issue time, so wave 1 is kept to 7 streams.
      - PE pre-warm: 11 dummy matmuls (~4.7us at the cold clock) end
        about when wave 1 lands, flipping the HAM clock gate to full
        speed with no idle gap (a >2us PE gap resets the clock).
      - No phases: tile k is a contiguous 16-matmul group gated on
        interleaved row-chunks of all four x slabs, so the PE never
        waits mid-tile and the clock never drops.
      - DVE drains each PSUM bank with a plain f32->bf16 copy; outputs
        stream out in 2-tile chunks with a split final chunk."""
    nc = bass.Bass()
    bf = mybir.dt.bfloat16
    f8 = mybir.dt.float8e4

    xa_d = nc.dram_tensor("xa", [128, FLAT], bf, kind="ExternalInput")
    x8_d = nc.dram_tensor("x8", [128, FLAT], f8, kind="ExternalInput")
    xb_d = nc.dram_tensor("xb", [128, FLAT], bf, kind="ExternalInput")
    xc_d = nc.dram_tensor("xc", [128, FLAT], bf, kind="ExternalInput")
    wt_d = nc.dram_tensor("wt", [128, 3 * NSLOT * 128], bf, kind="ExternalInput")
    w8_d = nc.dram_tensor("w8", [128, 3 * 2 * 256], f8, kind="ExternalInput")
    out_d = nc.dram_tensor("out", [128, 3 * HHALF * W], bf, kind="ExternalOutput")

    NPS = 8           # psum banks in rotation
    OCH = 2           # out-DMA granularity: blocks per chunk
    NT = 3 * NBLK     # 48 tiles
    WTM = NSLOT * 128  # wt cols per m-chunk
    # x row-chunk upper bounds (exclusive); tile k reads x rows <= 4*(k%16)+5,
    # so chunk c is first needed by tile FIRST_TILE[c]
    CH = [6, 14, 26, 38, 50, 62, 66]
    FIRST_TILE = [0, 1, 3, 6, 9, 12, 15]

    from contextlib import ExitStack
    with ExitStack() as ctx:
        xa_t = ctx.enter_context(nc.sbuf_tensor([128, FLAT], bf))
        x8_t = ctx.enter_context(nc.sbuf_tensor([128, FLAT], f8))
        xb_t = ctx.enter_context(nc.sbuf_tensor([128, FLAT], bf))
        xc_t = ctx.enter_context(nc.sbuf_tensor([128, FLAT], bf))
        wt_t = ctx.enter_context(nc.sbuf_tensor([128, 3 * NSLOT * 128], bf))
        w8_t = ctx.enter_context(nc.sbuf_tensor([128, 3 * 2 * 256], f8))
        st_t = ctx.enter_context(nc.sbuf_tensor([128, 3 * HHALF * W], bf))
        ps_t = ctx.enter_context(nc.psum_tensor([128, NPS * NFREE], mybir.dt.float32))
        dwt = ctx.enter_context(nc.semaphore("dwt"))
        dxa0 = ctx.enter_context(nc.semaphore("dxa0"))
        dxb0 = ctx.enter_context(nc.semaphore("dxb0"))
        dx = ctx.enter_context(nc.semaphore("dx"))
        dwtr = ctx.enter_context(nc.semaphore("dwtr"))
        pes = ctx.enter_context(nc.semaphore("pes"))
        dve = ctx.enter_context(nc.semaphore("dve"))
        dout = ctx.enter_context(nc.semaphore("dout"))
        warm = ctx.enter_context(nc.semaphore("warm"))
        block = ctx.enter_context(nc.Block())
        xa_v = xa_t[:].rearrange("p (r c) -> p r c", c=WP)
        xb_v = xb_t[:].rearrange("p (r c) -> p r c", c=WP)
        xc_v = xc_t[:].rearrange("p (r c) -> p r c", c=WP)

        def wt_ap(m, s):
            lo = (m * NSLOT + s) * 128
            return wt_t[:, lo:lo + 128]

        def w8_ap(m, d):
            lo = (m * 2 + d) * 256
            return w8_t[:, lo:lo + 256].rearrange("p (t q) -> p t q", t=2)

        def dr_mv(j0, j, h0=0, rsz=RB):
            # moving AP [p, t(2):WP, r(rsz):WP, c(W):1] at row j0+h0, col j
            return bass_rust.AP(
                x8_t[:].tensor, (j0 + h0) * WP + j,
                bass_rust.VecI64Pair(
                    [[FLAT, 128], [WP, 2], [WP, rsz], [1, W]]))

        QBANK = [7, 4, 5, 6]   # final-tile quarter -> psum bank

        def emit_tile_a(k, h0=0, rsz=RB, qb=None):
            # part 1: the 10 ci[0:128] slots (8 bf16 xa + 2 fp8 DR)
            m, blk = divmod(k, NBLK)
            j0 = blk * RB
            if qb is None:
                ps = ps_t[:, (k % NPS) * NFREE:(k % NPS + 1) * NFREE]
            else:
                ps = ps_t[:, qb * NFREE:qb * NFREE + rsz * W]
            for s, (kh, kw) in enumerate(TAPS_BF):
                nc.tensor.matmul(
                    ps, wt_ap(m, s),
                    xa_v[:, j0 + h0 + kh: j0 + h0 + kh + rsz, kw: kw + W],
                    start=(s == 0), stop=False)
            for d in DR_COLS:
                nc.tensor.matmul(ps, w8_ap(m, d), dr_mv(j0, d, h0=h0, rsz=rsz),
                                 start=False, stop=False, perf_mode=DRM)

        def emit_tile_b(k, h0=0, rsz=RB, qb=None):
            # part 2: the 6 ci[128:192] pair slots
            m, blk = divmod(k, NBLK)
            j0 = blk * RB
            if qb is None:
                ps = ps_t[:, (k % NPS) * NFREE:(k % NPS + 1) * NFREE]
            else:
                ps = ps_t[:, qb * NFREE:qb * NFREE + rsz * W]
            for i, (ta, _tb) in enumerate(PAIRS_XB):
                nc.tensor.matmul(
                    ps, wt_ap(m, 8 + i),
                    xb_v[:, j0 + h0 + ta[0]: j0 + h0 + ta[0] + rsz,
                         ta[1]: ta[1] + W],
                    start=False, stop=False)
            mm = nc.tensor.matmul(
                ps, wt_ap(m, 13),
                xc_v[:, j0 + h0 + PAIR_XC[0][0]: j0 + h0 + PAIR_XC[0][0] + rsz,
                     PAIR_XC[0][1]: PAIR_XC[0][1] + W],
                start=False, stop=True)
            mm.then_inc(pes, 1)

        def emit_tile(k):
            emit_tile_a(k)
            emit_tile_b(k)

        @block.sync
        def _(sync):
            def split2(dst, src, lo, hi, sem):
                mid = ((lo + hi) // 2 // 4) * 4
                sync.dma_start(dst[:, lo:mid], src[:, lo:mid]).then_inc(sem, 16)
                sync.dma_start(dst[:, mid:hi], src[:, mid:hi]).then_inc(sem, 16)

            def xchunk(c):
                lo = 0 if c == 0 else CH[c - 1] * WP
                hi = CH[c] * WP
                for dst, src in ((xa_t, xa_d), (xb_t, xb_d), (xc_t, xc_d),
                                 (x8_t, x8_d)):
                    sync.dma_start(dst[:, lo:hi], src[:, lo:hi]).then_inc(dx, 16)

            # wave 1, in issue-priority order (each dma_start costs ~0.6us
            # of Sync time, so order = priority): tile 0's first 10 matmuls
            # need only wt slots 0..7 (the first two thirds of the m0
            # chunk), w8-m0, and xa/x8 rows [0,6); its last 6 matmuls add
            # the pair-slot weights (third wt chunk) and xb/xc rows.
            W3 = WTM // 3 // 4 * 4
            c0 = CH[0] * WP
            sync.dma_start(wt_t[:, 0:W3], wt_d[:, 0:W3]).then_inc(dwt, 16)
            sync.dma_start(wt_t[:, W3:2 * W3], wt_d[:, W3:2 * W3]).then_inc(dwt, 16)
            sync.dma_start(xa_t[:, 0:c0], xa_d[:, 0:c0]).then_inc(dxa0, 16)
            sync.dma_start(x8_t[:, 0:c0], x8_d[:, 0:c0]).then_inc(dxa0, 16)
            sync.dma_start(w8_t[:, 0:512], w8_d[:, 0:512]).then_inc(dwt, 16)
            sync.dma_start(wt_t[:, 2 * W3:WTM], wt_d[:, 2 * W3:WTM]).then_inc(dxb0, 16)
            sync.dma_start(xb_t[:, 0:c0], xb_d[:, 0:c0]).then_inc(dxb0, 16)
            sync.dma_start(xc_t[:, 0:c0], xc_d[:, 0:c0]).then_inc(dxb0, 16)
            xchunk(1)
            # keep ~2 chunks in flight: issue chunk c+2 once chunk c landed
            sync.wait_ge(dxb0, 48)
            xchunk(2)
            for c in range(3, len(CH)):
                sync.wait_ge(dx, 64 * (c - 2))
                xchunk(c)
            # m1/m2 weights (needed from tile 16)
            split2(wt_t, wt_d, WTM, 2 * WTM, dwtr)
            split2(wt_t, wt_d, 2 * WTM, 3 * WTM, dwtr)
            sync.dma_start(w8_t[:, 512:1536], w8_d[:, 512:1536]).then_inc(dwtr, 16)
            # output chunks of OCH tiles; final chunk tapered: tile 46
            # whole, then tile 47 in four quarter-DMAs as each quarter's
            # epilogue lands
            nch = NT // OCH
            ninc = 0
            for c in range(nch):
                lo, hi = c * OCH * NFREE, (c + 1) * OCH * NFREE
                if c == nch - 1:
                    sync.wait_ge(dve, NT - 1)
                    mid = lo + NFREE
                    sync.dma_start(out_d[:, lo:mid], st_t[:, lo:mid]).then_inc(dout, 16)
                    QF = NFREE // 4
                    for q in range(4):
                        sync.wait_ge(dve, NT + q)
                        sync.dma_start(out_d[:, mid + q * QF:mid + (q + 1) * QF],
                                       st_t[:, mid + q * QF:mid + (q + 1) * QF]
                                       ).then_inc(dout, 16)
                    ninc += 5
                else:
                    sync.wait_ge(dve, OCH * (c + 1))
                    sync.dma_start(out_d[:, lo:hi], st_t[:, lo:hi]).then_inc(dout, 16)
                    ninc += 1
            sync.wait_ge(dout, 16 * ninc)

        @block.tensor
        def _(tensor):
            # pre-warm the PE HAM clock gate during the initial DMA wait:
            # ~11 dummy matmuls x ~427ns cold end about when wave 1 lands;
            # >3.4us of PE activity flips the clock to 2.4GHz. st_t is idle.
            tensor.wait_ge(warm, 1)
            for _ in range(12):
                nc.tensor.matmul(
                    ps_t[:, 7 * NFREE:8 * NFREE],
                    st_t[0:1, 0:128],
                    st_t[0:1, 0:NFREE],
                    start=True,
                    stop=True,
                )
            # tile 0 split-phase: its first 10 matmuls need only wt/w8 +
            # xa/x8 rows [0,6), which are DMA-issued ahead of xb/xc -- the
            # PE starts ~1.5us earlier and the clock never drops.
            tensor.wait_ge(dwt, 48)
            tensor.wait_ge(dxa0, 32)
            emit_tile_a(0)
            tensor.wait_ge(dxb0, 48)
            emit_tile_b(0)
            gates = {t: 64 * c for c, t in enumerate(FIRST_TILE) if c > 0}
            for k in range(1, NT - 1):
                if k in gates:
                    tensor.wait_ge(dx, gates[k])
                if k == 16:
                    tensor.wait_ge(dwtr, 80)
                # one bank-reuse wait covers 4 tiles: tiles k..k+3 need at
                # most dve >= k+3-(NPS-1), and DVE lags PE by well under
                # the 3-tile slack this leaves.
                if k >= NPS and (k - NPS) % 4 == 0:
                    tensor.wait_ge(dve, min(k + 3, NT - 1) - NPS + 1)
                emit_tile(k)
            # final tile split into four 1-row quarters in four different
            # banks: each quarter's epilogue+out-DMA overlaps the next
            # quarter's matmuls, and the last DMA is only 32KB
            for q in range(4):
                # QBANK[q] is free once its previous tile's copy landed:
                # bank 7 <- tile 39 (dve>=40, covered by the k=44 wait),
                # bank 4/5/6 <- tiles 44/45/46 (dve >= 45/46/47)
                if q > 0:
                    tensor.wait_ge(dve, NT - 4 + q)
                emit_tile_a(NT - 1, h0=q, rsz=1, qb=QBANK[q])
                emit_tile_b(NT - 1, h0=q, rsz=1, qb=QBANK[q])

        @block.vector
        def _(vector):
            nc.vector.memset(st_t[0:1, 0:NFREE], 0.0).then_inc(warm, 1)
            for k in range(NT - 1):
                ps = ps_t[:, (k % NPS) * NFREE:(k % NPS + 1) * NFREE]
                vector.wait_ge(pes, k + 1)
                nc.vector.tensor_copy(
                    st_t[:, k * NFREE:(k + 1) * NFREE], ps).then_inc(dve, 1)
            # final tile: four quarter-width copies matching the split groups
            k = NT - 1
            QF = NFREE // 4
            for q in range(4):
                ps_q = ps_t[:, QBANK[q] * NFREE:QBANK[q] * NFREE + QF]
                vector.wait_ge(pes, k + 1 + q)
                nc.vector.tensor_copy(
                    st_t[:, k * NFREE + q * QF:k * NFREE + (q + 1) * QF],
                    ps_q).then_inc(dve, 1)
    return nc


def _causal_mask():
    m = np.ones((KH, KW), dtype=np.float32)
    m[KH // 2, KW // 2:] = 0.0
    m[KH // 2 + 1:, :] = 0.0
    return m


def _prepare_in_maps(x, weight, bias, mask):
    # window-any of mask -> valid [B, H, W]
    ind = (np.asarray(mask)[:, 0] != 0)
    indp = np.zeros((B, H + 2 * PAD, W + 2 * PAD), dtype=bool)
    indp[:, PAD:PAD + H, PAD:PAD + W] = ind
    valid = np.zeros((B, H, W), dtype=bool)
    for dh in range(KH):
        for dw in range(KW):
            valid |= indp[:, dh:dh + H, dw:dw + W]

    w_sc = np.asarray(weight, dtype=np.float32) * _causal_mask()[None, None] * WSCALE
    w_bf = w_sc.astype(BF16)
    w_f8 = w_sc.astype(FP8)

    # bf16 weights: m-major image [128 K, 3 m-chunks x 14 slots x 128 couts]
    wt = np.zeros((3, NSLOT, 128, 128), dtype=BF16)
    for m in range(3):
        cl, ch = m * 128, (m + 1) * 128
        for s, (kh, kw) in enumerate(TAPS_BF):
            wt[m, s] = w_bf[cl:ch, 0:128, kh, kw].T
        for i, (ta, tb) in enumerate(PAIRS_XB):
            wt[m, 8 + i, 0:64] = w_bf[cl:ch, 128:192, ta[0], ta[1]].T
            wt[m, 8 + i, 64:128] = w_bf[cl:ch, 128:192, tb[0], tb[1]].T
        ta, tb = PAIR_XC
        wt[m, 13, 0:64] = w_bf[cl:ch, 128:192, ta[0], ta[1]].T
        wt[m, 13, 64:128] = w_bf[cl:ch, 128:192, tb[0], tb[1]].T
    wt_sb = np.ascontiguousarray(wt.transpose(2, 0, 1, 3)).reshape(128, 3 * NSLOT * 128)

    # fp8 DR weights: [128 K, 3 m x 2 slots x 2 t x 128 couts], t = kh
    w8 = np.zeros((3, 2, 2, 128, 128), dtype=FP8)
    for m in range(3):
        cl, ch = m * 128, (m + 1) * 128
        for d in DR_COLS:
            for t in range(2):
                w8[m, d, t] = w_f8[cl:ch, 0:128, t, d].T
    w8_sb = np.ascontiguousarray(w8.transpose(3, 0, 1, 2, 4)).reshape(128, 3 * 2 * 256)

    x_f32 = np.asarray(x, dtype=np.float32)
    x_bf = x_f32.astype(BF16)
    x_f8 = x_f32.astype(FP8)

    in_maps = []
    for c in range(NCORES):
        b, half = c // 2, c % 2
        r0 = half * HHALF
        lo = r0 - PAD
        src_lo = max(lo, 0)

        xp = np.zeros((128, NROWS, WP), dtype=BF16)
        xp[:, src_lo - lo:, PAD:PAD + W] = x_bf[b, 0:128, src_lo:r0 + HHALF, :]
        xp8 = np.zeros((128, NROWS, WP), dtype=FP8)
        xp8[:, src_lo - lo:, PAD:PAD + W] = x_f8[b, 0:128, src_lo:r0 + HHALF, :]

        x2p = np.zeros((64, NROWS, WP), dtype=BF16)
        x2p[:, src_lo - lo:, PAD:PAD + W] = x_bf[b, 128:192, src_lo:r0 + HHALF, :]
        x2 = x2p.reshape(64, FLAT)
        sh1 = np.zeros_like(x2)
        sh1[:, :-1] = x2[:, 1:]
        shr = np.zeros_like(x2)
        shr[:, :-WP] = x2[:, WP:]
        in_maps.append({
            "xa": np.ascontiguousarray(xp.reshape(128, FLAT)),
            "x8": np.ascontiguousarray(xp8.reshape(128, FLAT)),
            "xb": np.ascontiguousarray(np.concatenate([x2, sh1], axis=0)),
            "xc": np.ascontiguousarray(np.concatenate([x2, shr], axis=0)),
            "wt": wt_sb,
            "w8": w8_sb,
        })
    return in_maps, valid


def _assemble(results, valid, bias):
    bias_f = np.asarray(bias, dtype=np.float32)
    out_full = np.empty((B, COUT, H, W), dtype=np.float32)
    inv = np.float32(1.0 / WSCALE)
    for c in range(NCORES):
        b, half = c // 2, c % 2
        o = np.asarray(results[c]["out"]).astype(np.float32)
        o4 = o.reshape(128, 3, HHALF, W).transpose(1, 0, 2, 3).reshape(COUT, HHALF, W)
        o4 = o4 * inv + bias_f[:, None, None]
        v = valid[b, half * HHALF:(half + 1) * HHALF, :]
        out_full[b, :, half * HHALF:(half + 1) * HHALF, :] = np.where(v[None], o4, 0.0)
    return out_full


def kernel(x, weight, bias, mask, _trace=False):
    in_maps, valid = _prepare_in_maps(x, weight, bias, mask)
    nc = _build_program()
    res = run_bass_kernel_spmd(nc, in_maps, core_ids=list(range(NCORES)),
                               trace=_trace)
    out = _assemble(res.results, valid, bias)
    if _trace:
        return out, res
    return out


# revision 23
# speedup vs baseline: 1.1342x; 1.0007x over previous
"""Masked 5x5 conv (PixelCNN 'A' mask) on 8 Trainium2 NeuronCores.

Problem (hardcoded): x[4,192,128,128] f32, weight[384,192,5,5] f32,
bias[384] f32, mask[4,1,128,128] i32.
out = where(window_any(mask), conv(x, weight*maskA) + bias, 0).

The 'A' causal mask keeps 12 of 25 taps: rows kh=0,1 fully, row kh=2 only
kw=0,1 -- i.e. every tap reads the current output row or rows above it.

Sharding: core c = (batch b = c//2, row-half = c%2). Each core computes one
batch's 64 output rows for all 384 out channels (3 M=128 chunks).

Per output tile [128 cout, 4 rows x 128 cols = 512] we accumulate 16
matmuls into one PSUM bank (contraction 12 taps x 192 cin = 2304):
  - 8 bf16 taps x ci[0:128]            (tile xa)
  - 2 fp8e4 DoubleRow slots x ci[0:128]: taps (0,j)+(1,j) for j=0,1
    packed as K=256 (two k-tiles) via a [p, t:WP, r:WP, c:1] strided AP
    on the fp8 copy of xa -- each runs in the time of ONE bf16 matmul.
  - 5 bf16 tap-PAIRS x ci[128:192]     (tile xb: lower 64 partitions =
    ci[128:192] data, upper 64 = same data shifted 1 col)
  - 1 bf16 tap-pair (0,4)+(1,4) x ci[128:192] (tile xc: upper shifted 1 row)
All weights are pre-scaled x256 on host (exact in bf16; lifts the fp8
weights out of the e4m3 denormal range). PSUM holds 256*conv; the DVE
epilogue is a plain f32->bf16 copy, and the host applies /256 + bias and
the window-any(mask) zeroing in f32 during assembly.
"""

import numpy as np
import ml_dtypes

import bass_rust
import concourse.bass as bass
from concourse import mybir
from concourse.bass_utils import run_bass_kernel_spmd

B, CIN, COUT, H, W = 4, 192, 384, 128, 128
KH = KW = 5
PAD = 2
NCORES = 8
HHALF = 64          # output rows per core
NROWS = HHALF + 2   # input rows staged per core (2 above)
WP = W + 4          # padded width
FLAT = NROWS * WP   # 66*132 = 8712
RB = 4              # output rows per block
NBLK = HHALF // RB  # 16 blocks
NFREE = RB * W      # 512 = one PSUM bank of fp32
WSCALE = 256.0      # weight pre-scale (power of 2; undone on host)

# bf16 xa taps of the 'A' mask, ci[0:128] (slots 0..7)
TAPS_BF = [(0, 2), (0, 3), (0, 4), (1, 2), (1, 3), (1, 4), (2, 0), (2, 1)]
# fp8 DoubleRow slots: vertical tap pairs (0,j)+(1,j), ci[0:128]
DR_COLS = [0, 1]
# ci[128:192] handled as bf16 pairs packed into K=128 matmuls.
PAIRS_XB = [((0, 0), (0, 1)), ((0, 2), (0, 3)),
            ((1, 0), (1, 1)), ((1, 2), (1, 3)), ((2, 0), (2, 1))]
PAIR_XC = ((0, 4), (1, 4))
NSLOT = 14          # bf16 weight slots per m-chunk: 8 xa + 5 xb + 1 xc

BF16 = ml_dtypes.bfloat16
FP8 = ml_dtypes.float8_e4m3
DRM = mybir.MatmulPerfMode.DoubleRow


def _build_program():
    """Raw Bass (no Tile): this walrus build rejects instructions carrying
    more than ~1 embedded sync wait, so all synchronization is standalone
    wait_ge instructions with manually-managed semaphores.

    Schedule (per core, ~180us):
      - The framework preamble holds every engine until ~8us; the first
        DMA cannot issue before ~7.5us and each dma_start costs ~0.6us of
        Sync issue time, so wave 1 is kept to 7 streams.
      - PE pre-warm: 11 dummy matmuls (~4.7us at the cold clock) end
        about when wave 1 lands, flipping the HAM clock gate to full
        speed with no idle gap (a >2us PE gap resets the clock).
      - No phases: tile k is a contiguous 16-matmul group gated on
        interleaved row-chunks of all four x slabs, so the PE never
        waits mid-tile and the clock never drops.
      - DVE drains each PSUM bank with a plain f32->bf16 copy; outputs
        stream out in 2-tile chunks with a split final chunk."""
    nc = bass.Bass()
    bf = mybir.dt.bfloat16
    f8 = mybir.dt.float8e4

    xa_d = nc.dram_tensor("xa", [128, FLAT], bf, kind="ExternalInput")
    x8_d = nc.dram_tensor("x8", [128, FLAT], f8, kind="ExternalInput")
    xb_d = nc.dram_tensor("xb", [128, FLAT], bf, kind="ExternalInput")
    xc_d = nc.dram_tensor("xc", [128, FLAT], bf, kind="ExternalInput")
    wt_d = nc.dram_tensor("wt", [128, 3 * NSLOT * 128], bf, kind="ExternalInput")
    w8_d = nc.dram_tensor("w8", [128, 3 * 2 * 256], f8, kind="ExternalInput")
    out_d = nc.dram_tensor("out", [128, 3 * HHALF * W], bf, kind="ExternalOutput")

    NPS = 8           # psum banks in rotation
    OCH = 4           # out-DMA granularity: blocks per chunk
    NT = 3 * NBLK     # 48 tiles
    WTM = NSLOT * 128  # wt cols per m-chunk
    # x row-chunk upper bounds (exclusive); tile k reads x rows <= 4*(k%16)+5,
    # so chunk c is first needed by tile FIRST_TILE[c]
    CH = [6, 14, 26, 38, 50, 62, 66]
    FIRST_TILE = [0, 1, 3, 6, 9, 12, 15]

    from contextlib import ExitStack
    with ExitStack() as ctx:
        xa_t = ctx.enter_context(nc.sbuf_tensor([128, FLAT], bf))
        x8_t = ctx.enter_context(nc.sbuf_tensor([128, FLAT], f8))
        xb_t = ctx.enter_context(nc.sbuf_tensor([128, FLAT], bf))
        xc_t = ctx.enter_context(nc.sbuf_tensor([128, FLAT], bf))
        wt_t = ctx.enter_context(nc.sbuf_tensor([128, 3 * NSLOT * 128], bf))
        w8_t = ctx.enter_context(nc.sbuf_tensor([128, 3 * 2 * 256], f8))
        st_t = ctx.enter_context(nc.sbuf_tensor([128, 3 * HHALF * W], bf))
        ps_t = ctx.enter_context(nc.psum_tensor([128, NPS * NFREE], mybir.dt.float32))
        dwt = ctx.enter_context(nc.semaphore("dwt"))
        dxa0 = ctx.enter_context(nc.semaphore("dxa0"))
        dxb0 = ctx.enter_context(nc.semaphore("dxb0"))
        dx = ctx.enter_context(nc.semaphore("dx"))
        dwtr = ctx.enter_context(nc.semaphore("dwtr"))
        pes = ctx.enter_context(nc.semaphore("pes"))
        dve = ctx.enter_context(nc.semaphore("dve"))
        dout = ctx.enter_context(nc.semaphore("dout"))
        warm = ctx.enter_context(nc.semaphore("warm"))
        block = ctx.enter_context(nc.Block())
        xa_v = xa_t[:].rearrange("p (r c) -> p r c", c=WP)
        xb_v = xb_t[:].rearrange("p (r c) -> p r c", c=WP)
        xc_v = xc_t[:].rearrange("p (r c) -> p r c", c=WP)

        def wt_ap(m, s):
            lo = (m * NSLOT + s) * 128
            return wt_t[:, lo:lo + 128]

        def w8_ap(m, d):
            lo = (m * 2 + d) * 256
            return w8_t[:, lo:lo + 256].rearrange("p (t q) -> p t q", t=2)

        def dr_mv(j0, j, h0=0, rsz=RB):
            # moving AP [p, t(2):WP, r(rsz):WP, c(W):1] at row j0+h0, col j
            return bass_rust.AP(
                x8_t[:].tensor, (j0 + h0) * WP + j,
                bass_rust.VecI64Pair(
                    [[FLAT, 128], [WP, 2], [WP, rsz], [1, W]]))

        QBANK = [7, 4, 5, 6]   # final-tile quarter -> psum bank

        def emit_tile_a(k, h0=0, rsz=RB, qb=None):
            # part 1: the 10 ci[0:128] slots (8 bf16 xa + 2 fp8 DR)
            m, blk = divmod(k, NBLK)
            j0 = blk * RB
            if qb is None:
                ps = ps_t[:, (k % NPS) * NFREE:(k % NPS + 1) * NFREE]
            else:
                ps = ps_t[:, qb * NFREE:qb * NFREE + rsz * W]
            for s, (kh, kw) in enumerate(TAPS_BF):
                nc.tensor.matmul(
                    ps, wt_ap(m, s),
                    xa_v[:, j0 + h0 + kh: j0 + h0 + kh + rsz, kw: kw + W],
                    start=(s == 0), stop=False)
            for d in DR_COLS:
                nc.tensor.matmul(ps, w8_ap(m, d), dr_mv(j0, d, h0=h0, rsz=rsz),
                                 start=False, stop=False, perf_mode=DRM)

        def emit_tile_b(k, h0=0, rsz=RB, qb=None):
            # part 2: the 6 ci[128:192] pair slots
            m, blk = divmod(k, NBLK)
            j0 = blk * RB
            if qb is None:
                ps = ps_t[:, (k % NPS) * NFREE:(k % NPS + 1) * NFREE]
            else:
                ps = ps_t[:, qb * NFREE:qb * NFREE + rsz * W]
            for i, (ta, _tb) in enumerate(PAIRS_XB):
                nc.tensor.matmul(
                    ps, wt_ap(m, 8 + i),
                    xb_v[:, j0 + h0 + ta[0]: j0 + h0 + ta[0] + rsz,
                         ta[1]: ta[1] + W],
                    start=False, stop=False)
            mm = nc.tensor.matmul(
                ps, wt_ap(m, 13),
                xc_v[:, j0 + h0 + PAIR_XC[0][0]: j0 + h0 + PAIR_XC[0][0] + rsz,
                     PAIR_XC[0][1]: PAIR_XC[0][1] + W],
                start=False, stop=True)
            mm.then_inc(pes, 1)

        def emit_tile(k):
            emit_tile_a(k)
            emit_tile_b(k)

        @block.sync
        def _(sync):
            def split2(dst, src, lo, hi, sem):
                mid = ((lo + hi) // 2 // 4) * 4
                sync.dma_start(dst[:, lo:mid], src[:, lo:mid]).then_inc(sem, 16)
                sync.dma_start(dst[:, mid:hi], src[:, mid:hi]).then_inc(sem, 16)

            def xchunk(c):
                lo = 0 if c == 0 else CH[c - 1] * WP
                hi = CH[c] * WP
                for dst, src in ((xa_t, xa_d), (xb_t, xb_d), (xc_t, xc_d),
                                 (x8_t, x8_d)):
                    sync.dma_start(dst[:, lo:hi], src[:, lo:hi]).then_inc(dx, 16)

            # wave 1, in issue-priority order (each dma_start costs ~0.6us
            # of Sync time, so order = priority): tile 0's first 10 matmuls
            # need only wt slots 0..7 (the first two thirds of the m0
            # chunk), w8-m0, and xa/x8 rows [0,6); its last 6 matmuls add
            # the pair-slot weights (third wt chunk) and xb/xc rows.
            W3 = WTM // 3 // 4 * 4
            c0 = CH[0] * WP
            sync.dma_start(wt_t[:, 0:W3], wt_d[:, 0:W3]).then_inc(dwt, 16)
            sync.dma_start(wt_t[:, W3:2 * W3], wt_d[:, W3:2 * W3]).then_inc(dwt, 16)
            sync.dma_start(xa_t[:, 0:c0], xa_d[:, 0:c0]).then_inc(dxa0, 16)
            sync.dma_start(x8_t[:, 0:c0], x8_d[:, 0:c0]).then_inc(dxa0, 16)
            sync.dma_start(w8_t[:, 0:512], w8_d[:, 0:512]).then_inc(dwt, 16)
            sync.dma_start(wt_t[:, 2 * W3:WTM], wt_d[:, 2 * W3:WTM]).then_inc(dxb0, 16)
            sync.dma_start(xb_t[:, 0:c0], xb_d[:, 0:c0]).then_inc(dxb0, 16)
            sync.dma_start(xc_t[:, 0:c0], xc_d[:, 0:c0]).then_inc(dxb0, 16)
            xchunk(1)
            # keep ~2 chunks in flight: issue chunk c+2 once chunk c landed
            sync.wait_ge(dxb0, 48)
            xchunk(2)
            for c in range(3, len(CH)):
                sync.wait_ge(dx, 64 * (c - 2))
                xchunk(c)
            # m1/m2 weights (needed from tile 16)
            split2(wt_t, wt_d, WTM, 2 * WTM, dwtr)
            split2(wt_t, wt_d, 2 * WTM, 3 * WTM, dwtr)
            sync.dma_start(w8_t[:, 512:1536], w8_d[:, 512:1536]).then_inc(dwtr, 16)
            # output: tiles 0..43 in chunks of OCH=4 (fewer, bigger DMAs
            # contend less with the PE's SBUF reads), then a tapered tail:
            # tiles 44-45 paired, 46 alone, 47 as four quarter-DMAs
            for c in range(11):
                lo, hi = c * OCH * NFREE, (c + 1) * OCH * NFREE
                sync.wait_ge(dve, OCH * (c + 1))
                sync.dma_start(out_d[:, lo:hi], st_t[:, lo:hi]).then_inc(dout, 16)
            sync.wait_ge(dve, 46)
            sync.dma_start(out_d[:, 44 * NFREE:46 * NFREE],
                           st_t[:, 44 * NFREE:46 * NFREE]).then_inc(dout, 16)
            sync.wait_ge(dve, 47)
            sync.dma_start(out_d[:, 46 * NFREE:47 * NFREE],
                           st_t[:, 46 * NFREE:47 * NFREE]).then_inc(dout, 16)
            QF = NFREE // 4
            base = 47 * NFREE
            for q in range(4):
                sync.wait_ge(dve, NT + q)
                sync.dma_start(out_d[:, base + q * QF:base + (q + 1) * QF],
                               st_t[:, base + q * QF:base + (q + 1) * QF]
                               ).then_inc(dout, 16)
            sync.wait_ge(dout, 16 * 17)

        @block.tensor
        def _(tensor):
            # pre-warm the PE HAM clock gate during the initial DMA wait.
            # The dummies must be FULL-K (128 partitions): K=1 dummies draw
            # no array power and never flip the clock -- the first ~7
            # full-K matmuls run at ~427ns (low pstate), then ~216ns. 17
            # of them end about when wave 1 lands; any idle gap >~1us
            # drops the clock again. st_t garbage is fine: bank 7's first
            # real user (tile 7) starts its group with start=True.
            tensor.wait_ge(warm, 1)
            for _ in range(17):
                nc.tensor.matmul(
                    ps_t[:, 7 * NFREE:8 * NFREE],
                    st_t[:, 0:128],
                    st_t[:, 0:NFREE],
                    start=True,
                    stop=True,
                )
            # tile 0 split-phase: its first 10 matmuls need only wt/w8 +
            # xa/x8 rows [0,6), which are DMA-issued ahead of xb/xc -- the
            # PE starts ~1.5us earlier and the clock never drops.
            tensor.wait_ge(dwt, 48)
            tensor.wait_ge(dxa0, 32)
            emit_tile_a(0)
            tensor.wait_ge(dxb0, 48)
            emit_tile_b(0)
            gates = {t: 64 * c for c, t in enumerate(FIRST_TILE) if c > 0}
            for k in range(1, NT - 1):
                if k in gates:
                    tensor.wait_ge(dx, gates[k])
                if k == 16:
                    tensor.wait_ge(dwtr, 80)
                # one bank-reuse wait covers 4 tiles: tiles k..k+3 need at
                # most dve >= k+3-(NPS-1), and DVE lags PE by well under
                # the 3-tile slack this leaves.
                if k >= NPS and (k - NPS) % 4 == 0:
                    tensor.wait_ge(dve, min(k + 3, NT - 1) - NPS + 1)
                emit_tile(k)
            # final tile split into four 1-row quarters in four different
            # banks: each quarter's epilogue+out-DMA overlaps the next
            # quarter's matmuls, and the last DMA is only 32KB
            for q in range(4):
                # QBANK[q] is free once its previous tile's copy landed:
                # bank 7 <- tile 39 (dve>=40, covered by the k=44 wait),
                # bank 4/5/6 <- tiles 44/45/46 (dve >= 45/46/47)
                if q > 0:
                    tensor.wait_ge(dve, NT - 4 + q)
                emit_tile_a(NT - 1, h0=q, rsz=1, qb=QBANK[q])
                emit_tile_b(NT - 1, h0=q, rsz=1, qb=QBANK[q])

        @block.vector
        def _(vector):
            nc.vector.memset(st_t[0:1, 0:NFREE], 0.0).then_inc(warm, 1)
            for k in range(NT - 1):
                ps = ps_t[:, (k % NPS) * NFREE:(k % NPS + 1) * NFREE]
                vector.wait_ge(pes, k + 1)
                nc.vector.tensor_copy(
                    st_t[:, k * NFREE:(k + 1) * NFREE], ps).then_inc(dve, 1)
            # final tile: four quarter-width copies matching the split groups
            k = NT - 1
            QF = NFREE // 4
            for q in range(4):
                ps_q = ps_t[:, QBANK[q] * NFREE:QBANK[q] * NFREE + QF]
                vector.wait_ge(pes, k + 1 + q)
                nc.vector.tensor_copy(
                    st_t[:, k * NFREE + q * QF:k * NFREE + (q + 1) * QF],
                    ps_q).then_inc(dve, 1)
    return nc


def _causal_mask():
    m = np.ones((KH, KW), dtype=np.float32)
    m[KH // 2, KW // 2:] = 0.0
    m[KH // 2 + 1:, :] = 0.0
    return m


def _prepare_in_maps(x, weight, bias, mask):
    # window-any of mask -> valid [B, H, W]
    ind = (np.asarray(mask)[:, 0] != 0)
    indp = np.zeros((B, H + 2 * PAD, W + 2 * PAD), dtype=bool)
    indp[:, PAD:PAD + H, PAD:PAD + W] = ind
    valid = np.zeros((B, H, W), dtype=bool)
    for dh in range(KH):
        for dw in range(KW):
            valid |= indp[:, dh:dh + H, dw:dw + W]

    w_sc = np.asarray(weight, dtype=np.float32) * _causal_mask()[None, None] * WSCALE
    w_bf = w_sc.astype(BF16)
    w_f8 = w_sc.astype(FP8)

    # bf16 weights: m-major image [128 K, 3 m-chunks x 14 slots x 128 couts]
    wt = np.zeros((3, NSLOT, 128, 128), dtype=BF16)
    for m in range(3):
        cl, ch = m * 128, (m + 1) * 128
        for s, (kh, kw) in enumerate(TAPS_BF):
            wt[m, s] = w_bf[cl:ch, 0:128, kh, kw].T
        for i, (ta, tb) in enumerate(PAIRS_XB):
            wt[m, 8 + i, 0:64] = w_bf[cl:ch, 128:192, ta[0], ta[1]].T
            wt[m, 8 + i, 64:128] = w_bf[cl:ch, 128:192, tb[0], tb[1]].T
        ta, tb = PAIR_XC
        wt[m, 13, 0:64] = w_bf[cl:ch, 128:192, ta[0], ta[1]].T
        wt[m, 13, 64:128] = w_bf[cl:ch, 128:192, tb[0], tb[1]].T
    wt_sb = np.ascontiguousarray(wt.transpose(2, 0, 1, 3)).reshape(128, 3 * NSLOT * 128)

    # fp8 DR weights: [128 K, 3 m x 2 slots x 2 t x 128 couts], t = kh
    w8 = np.zeros((3, 2, 2, 128, 128), dtype=FP8)
    for m in range(3):
        cl, ch = m * 128, (m + 1) * 128
        for d in DR_COLS:
            for t in range(2):
                w8[m, d, t] = w_f8[cl:ch, 0:128, t, d].T
    w8_sb = np.ascontiguousarray(w8.transpose(3, 0, 1, 2, 4)).reshape(128, 3 * 2 * 256)

    x_f32 = np.asarray(x, dtype=np.float32)
    x_bf = x_f32.astype(BF16)
    x_f8 = x_f32.astype(FP8)

    in_maps = []
    for c in range(NCORES):
        b, half = c // 2, c % 2
        r0 = half * HHALF
        lo = r0 - PAD
        src_lo = max(lo, 0)

        xp = np.zeros((128, NROWS, WP), dtype=BF16)
        xp[:, src_lo - lo:, PAD:PAD + W] = x_bf[b, 0:128, src_lo:r0 + HHALF, :]
        xp8 = np.zeros((128, NROWS, WP), dtype=FP8)
        xp8[:, src_lo - lo:, PAD:PAD + W] = x_f8[b, 0:128, src_lo:r0 + HHALF, :]

        x2p = np.zeros((64, NROWS, WP), dtype=BF16)
        x2p[:, src_lo - lo:, PAD:PAD + W] = x_bf[b, 128:192, src_lo:r0 + HHALF, :]
        x2 = x2p.reshape(64, FLAT)
        sh1 = np.zeros_like(x2)
        sh1[:, :-1] = x2[:, 1:]
        shr = np.zeros_like(x2)
        shr[:, :-WP] = x2[:, WP:]
        in_maps.append({
            "xa": np.ascontiguousarray(xp.reshape(128, FLAT)),
            "x8": np.ascontiguousarray(xp8.reshape(128, FLAT)),
            "xb": np.ascontiguousarray(np.concatenate([x2, sh1], axis=0)),
            "xc": np.ascontiguousarray(np.concatenate([x2, shr], axis=0)),
            "wt": wt_sb,
            "w8": w8_sb,
        })
    return in_maps, valid


def _assemble(results, valid, bias):
    bias_f = np.asarray(bias, dtype=np.float32)
    out_full = np.empty((B, COUT, H, W), dtype=np.float32)
    inv = np.float32(1.0 / WSCALE)
    for c in range(NCORES):
        b, half = c // 2, c % 2
        o = np.asarray(results[c]["out"]).astype(np.float32)
        o4 = o.reshape(128, 3, HHALF, W).transpose(1, 0, 2, 3).reshape(COUT, HHALF, W)
        o4 = o4 * inv + bias_f[:, None, None]
        v = valid[b, half * HHALF:(half + 1) * HHALF, :]
        out_full[b, :, half * HHALF:(half + 1) * HHALF, :] = np.where(v[None], o4, 0.0)
    return out_full


def kernel(x, weight, bias, mask, _trace=False):
    in_maps, valid = _prepare_in_maps(x, weight, bias, mask)
    nc = _build_program()
    res = run_bass_kernel_spmd(nc, in_maps, core_ids=list(range(NCORES)),
                               trace=_trace)
    out = _assemble(res.results, valid, bias)
    if _trace:
        return out, res
    return out
